# revision 1
# baseline (speedup 1.0000x reference)
"""Trainium2 8-core kernel for nn_Attention_27530740367526.

Multi-head causal attention (B=2, S=2048, D=2048, H=16, HD=128, fp32) with
RoPE, sharded batch x head-group across 8 NeuronCores: core c handles batch
c//4 and heads [4*(c%4), 4*(c%4)+4).  Each core computes q/k/v projections
(+RoPE), attention for its heads, and the slice of the wo projection those
heads feed — a partial [S, D] output.  The host sums the 4 partials per
batch (the row-parallel wo "all-reduce" is a host-side unshard).

On-device everything lives in "transposed land": qT/kT are [head_dim, seq]
with head-dim on partitions, so scores come out transposed ([k, q]), the
softmax denominator is an all-ones-column matmul (giving a partition-
broadcast denominator for free), and PV / wo consume natural layouts with
zero on-device transposes.  RoPE's rotate-half is a 128x128 permutation
matmul on the PE.  All matmul operands are float32r (fp32 rounded to 11
explicit mantissa bits, pre-rounded bit-exactly on the host) which runs at
full PE rate.

The kernel is fully fused: heads are processed in pairs (so all weights +
k/v stay in SBUF), and within a pair the work is streamed per 512-column
sequence chunk — project q/k/v for the chunk, run causal attention for
that query chunk against all earlier k/v chunks (available because
processing is in causal order), and emit the previous chunk's slice of the
wo projection as PE filler work inside the attention's softmax-wait
bubbles.  The second head-pair accumulates into the output via CCE
DMA-add.
"""

import sys

if "/opt/trn_rl_repo" not in sys.path:
    sys.path.insert(0, "/opt/trn_rl_repo")

from collections import deque

import numpy as np

import concourse.bacc as bacc
import concourse.mybir as mybir
import concourse.tile as tile
from concourse.bass_utils import run_bass_kernel_spmd

F32 = mybir.dt.float32
F32R = mybir.dt.float32r
AF = mybir.ActivationFunctionType

N_HEADS = 16
N_CORES = 8
B, S, D = 2, 2048, 2048
HD = D // N_HEADS
H_LOC = N_HEADS // (N_CORES // B)  # 4 heads per core
HW = H_LOC * HD                    # 512 q/k/v columns per core
SC = 512                           # seq chunk (matmul moving free dim)
P = 128
LOOKAHEAD = 3                      # scores-tile software pipeline depth


def _round_f32r(x: np.ndarray) -> np.ndarray:
    """Host-side fp32 -> float32r rounding (RNE to 11 explicit mantissa
    bits); bit-exact with the device DVE rounding."""
    xi = np.ascontiguousarray(x, dtype=np.float32).view(np.uint32)
    nbits = 12
    lo = np.uint32((1 << nbits) - 1)
    half = np.uint32(1 << (nbits - 1))
    rem = xi & lo
    up = (rem > half) | ((rem == half) & (((xi >> nbits) & 1) == 1))
    r = (xi & ~lo) + np.where(up, np.uint32(1 << nbits), np.uint32(0))
    return r.view(np.float32)


def _build_core_kernel(causal: bool):
    KO = D // P            # 16 contraction subtiles for projections
    NQC = S // SC          # 4 q-chunks
    NSUB = SC // P         # 4 128-blocks per chunk
    NST = S // P           # 16 s-tiles
    NHB = H_LOC // 2       # head pairs
    inv_sqrt_hd = 1.0 / float(np.sqrt(HD))

    nc = bacc.Bacc(None, target_bir_lowering=False)

    xT = nc.dram_tensor("xT", [D, S], F32R, kind="ExternalInput")
    wqkvT = nc.dram_tensor(
        "wqkvT", [H_LOC // 2, D, 6 * HD], F32R, kind="ExternalInput"
    )
    woT = nc.dram_tensor("woT", [HW, D], F32R, kind="ExternalInput")
    cosT = nc.dram_tensor("cosT", [HD, S], F32, kind="ExternalInput")
    sinT = nc.dram_tensor("sinT", [HD, S], F32, kind="ExternalInput")
    PT = nc.dram_tensor("PT", [HD, HD], F32R, kind="ExternalInput")
    ones = nc.dram_tensor("ones", [P, P], F32R, kind="ExternalInput")
    if causal:
        maskT = nc.dram_tensor("maskT", [SC, SC], F32, kind="ExternalInput")
    else:
        maskT = nc.dram_tensor("maskT", [S, S], F32, kind="ExternalInput")
    y = nc.dram_tensor("y", [S, D], F32, kind="ExternalOutput")

    xT_r = xT.rearrange("(ko ki) s -> ki ko s", ki=P)
    wqkvT_r = wqkvT.rearrange("hb (ko ki) c -> hb ki ko c", ki=P)
    woT_r = woT.rearrange("(h ki) d -> ki h d", ki=P)

    with tile.TileContext(nc) as tc:
        with (
            tc.tile_pool(name="persist", bufs=1) as persist,
            tc.tile_pool(name="wpool", bufs=1) as wpool,
            tc.tile_pool(name="kvq", bufs=1) as kvq,
            tc.tile_pool(name="xa", bufs=1) as xa,
            tc.tile_pool(name="cs", bufs=2) as cspool,
            tc.tile_pool(name="scr", bufs=2) as scr,
            tc.tile_pool(name="exps", bufs=4) as expp,
            tc.tile_pool(name="outq", bufs=2) as outqp,
            tc.tile_pool(name="yo", bufs=4) as yop,
            tc.tile_pool(name="gm", bufs=3) as gmp,
            tc.tile_pool(name="ps", bufs=3, space="PSUM") as cyc,
            tc.tile_pool(name="ops", bufs=2, space="PSUM") as ops,
            tc.tile_pool(name="dps", bufs=2, space="PSUM") as dps,
            tc.tile_pool(name="yps", bufs=1, space="PSUM") as yps,
        ):
            pt_sb = persist.tile([P, HD], F32R)
            nc.sync.dma_start(pt_sb[:], PT[:])
            ones_sb = persist.tile([P, P], F32R)
            nc.sync.dma_start(ones_sb[:], ones[:])
            if causal:
                mask_sb = persist.tile([P, NSUB, SC], F32)
                nc.sync.dma_start(
                    mask_sb[:], maskT.rearrange("(j ki) q -> ki j q", ki=P)
                )

            def load_chunk(sc):
                ssl = slice(sc * SC, (sc + 1) * SC)
                xt = xa.tile([P, KO, SC], F32R, tag="xt")
                for ko in range(KO):
                    nc.sync.dma_start(xt[:, ko], xT_r[:, ko, ssl])
                cos_t = cspool.tile([P, SC], F32, tag="cos")
                sin_t = cspool.tile([P, SC], F32, tag="sin")
                nc.sync.dma_start(cos_t[:], cosT[:, ssl])
                nc.sync.dma_start(sin_t[:], sinT[:, ssl])
                return xt, cos_t, sin_t

            preloaded = None
            for hb in range(NHB):
                if hb == 0:
                    # queue the first x-chunk's DMAs ahead of the (large)
                    # weight loads so the PE can start within a few us
                    preloaded = load_chunk(0)
                # ---- weights for this head pair, one DMA per ko slice
                # (host pre-packs the pair's q/k/v columns contiguously so
                #  every descriptor is a 3 KB row)
                w_sb = wpool.tile([P, KO, 6 * HD], F32R, tag="w")
                for ko in range(KO):
                    nc.sync.dma_start(
                        w_sb[:, ko, :], wqkvT_r[hb, :, ko, :]
                    )
                wo_sb = wpool.tile([P, 2, D], F32R, tag="wo")
                for hl in range(2):
                    nc.sync.dma_start(
                        wo_sb[:, hl], woT_r[:, hb * 2 + hl, :]
                    )

                # ---- per-pair persistent qkv ----
                kT_sb = kvq.tile([P, 2, S], F32R, tag="kT")
                v_sb = kvq.tile([P, NST, 2 * HD], F32R, tag="v")
                deferred = not causal
                qT_full = (
                    kvq.tile([P, 2, S], F32R, tag="qT", name="qT_full") if deferred else None
                )

                def project_chunk(sc, loaded):
                    if deferred:
                        qT_dst = qT_full
                    else:
                        qT_dst = outqp.tile([P, 2, SC], F32R, tag="qTc")
                    ssl = slice(sc * SC, (sc + 1) * SC)
                    xt, cos_t, sin_t = loaded

                    for hl in range(2):
                        for t in range(2):  # 0=q, 1=k
                            wcols = slice(
                                (2 * hl + t) * HD, (2 * hl + t + 1) * HD
                            )
                            ps = cyc.tile([P, SC], F32, tag="ps")
                            for ko in range(KO):
                                nc.tensor.matmul(
                                    ps[:],
                                    w_sb[:, ko, wcols],
                                    xt[:, ko],
                                    start=(ko == 0),
                                    stop=(ko == KO - 1),
                                )
                            plain = scr.tile([P, SC], F32R, tag="plain")
                            nc.scalar.copy(plain[:], ps[:])
                            rot = cyc.tile([P, SC], F32, tag="ps")
                            nc.tensor.matmul(rot[:], pt_sb[:], plain[:])
                            if t == 0:
                                dst = (
                                    qT_dst[:, hl, ssl]
                                    if deferred
                                    else qT_dst[:, hl, :]
                                )
                            else:
                                dst = kT_sb[:, hl, ssl]
                            # rope: dst = plain*cos + rot*sin
                            # (final add on DVE performs the f32r rounding)
                            pc = scr.tile([P, SC], F32, tag="pc")
                            nc.gpsimd.tensor_mul(pc[:], plain[:], cos_t[:])
                            tmp2 = scr.tile([P, SC], F32, tag="tmp2")
                            nc.vector.tensor_mul(tmp2[:], rot[:], sin_t[:])
                            nc.vector.tensor_add(dst, pc[:], tmp2[:])
                            # (rope-mul engine choice patched in bisection)

                    for sti in range(NSUB):
                        st = sc * NSUB + sti
                        lsl = slice(sti * P, (sti + 1) * P)
                        psv = cyc.tile([P, 2 * HD], F32, tag="ps")
                        for ko in range(KO):
                            nc.tensor.matmul(
                                psv[:],
                                xt[:, ko, lsl],
                                w_sb[:, ko, 4 * HD : 6 * HD],
                                start=(ko == 0),
                                stop=(ko == KO - 1),
                            )
                        vdst = v_sb[:, st, :]
                        nc.scalar.copy(vdst, psv[:])
                    return qT_dst

                def attend_chunk(qc, qT_cur, outT_qc, fillers):
                    """Attention for query chunk qc, both heads of the pair
                    interleaved per k-block (so the PE always has two
                    independent softmax chains in flight), writing
                    normalized outT [hd, q] slices.  `fillers` is a deque
                    of independent PE-work closures drained into the
                    pipeline's tail bubbles."""
                    nkb = (qc + 1) * NSUB if causal else NST
                    qt = {}
                    o_ps = {}
                    d_ps = {}
                    for hl in range(2):
                        qt[hl] = (
                            qT_cur[:, hl, qc * SC : (qc + 1) * SC]
                            if deferred
                            else qT_cur[:, hl, :]
                        )
                        o_ps[hl] = ops.tile([P, SC], F32, tag="o", name=f"o_ps{hl}")
                        d_ps[hl] = dps.tile([P, SC], F32, tag="d", name=f"d_ps{hl}")
                    stile = {}

                    def emit_scores(kb, hl):
                        t_ = cyc.tile([P, SC], F32, tag="ps")
                        nc.tensor.matmul(
                            t_[:],
                            kT_sb[:, hl, kb * P : (kb + 1) * P],
                            qt[hl],
                            skip_group_check=True,
                        )
                        if causal:
                            j = kb - qc * NSUB
                            if j >= 0:
                                w_ = P * (j + 1)
                                nc.vector.tensor_add(
                                    t_[:, :w_], t_[:, :w_],
                                    mask_sb[:, j, :w_],
                                )
                        else:
                            if hl == 0:
                                mt = gmp.tile([P, SC], F32, tag="mt")
                                nc.sync.dma_start(
                                    mt[:],
                                    maskT[
                                        kb * P : (kb + 1) * P,
                                        qc * SC : (qc + 1) * SC,
                                    ],
                                )
                                stile[("m", kb)] = mt
                            nc.vector.tensor_add(
                                t_[:], t_[:], stile[("m", kb)][:]
                            )
                        stile[(kb, hl)] = t_

                    # lookahead in (kb, hl) steps; 3 PSUM slots for scores
                    seq = [(kb, hl) for kb in range(nkb) for hl in range(2)]
                    for kb, hl in seq[:LOOKAHEAD]:
                        emit_scores(kb, hl)
                    for i, (kb, hl) in enumerate(seq):
                        e = expp.tile([P, SC], F32R, tag="e")
                        nc.scalar.activation(
                            e[:], stile.pop((kb, hl))[:], AF.Exp,
                            scale=inv_sqrt_hd,
                        )
                        nc.tensor.matmul(
                            o_ps[hl][:],
                            v_sb[:, kb, hl * HD : (hl + 1) * HD],
                            e[:],
                            start=(kb == 0),
                            stop=(kb == nkb - 1),
                            skip_group_check=True,
                        )
                        nc.tensor.matmul(
                            d_ps[hl][:],
                            ones_sb[:],
                            e[:],
                            start=(kb == 0),
                            stop=(kb == nkb - 1),
                            skip_group_check=True,
                        )
                        if i + LOOKAHEAD < len(seq):
                            emit_scores(*seq[i + LOOKAHEAD])
                            if fillers and i % 2 == 1:
                                fillers.popleft()()
                        elif fillers:
                            fillers.popleft()()
                    for hl in range(2):
                        recip = scr.tile([P, SC], F32, tag="recip")
                        nc.vector.reciprocal_approx_fast(
                            recip[:], d_ps[hl][:]
                        )
                        nc.vector.tensor_mul(
                            outT_qc[:, hl, :], o_ps[hl][:], recip[:]
                        )

                def make_out_fillers(hb, qc, outT_qc):
                    """One closure per (s-tile, d-chunk) block of the wo
                    projection for query chunk qc: 2 accumulating matmuls,
                    a PSUM->SBUF copy (alternating DVE/ACT), and the output
                    DMA (plain HWDGE write for pair 0, CCE accumulate for
                    pair 1)."""
                    work = []
                    for sti in range(NSUB):
                        st = qc * NSUB + sti
                        stsl = slice(sti * P, (sti + 1) * P)
                        for dc in range(D // SC):
                            dsl = slice(dc * SC, (dc + 1) * SC)

                            def blk(st=st, stsl=stsl, dsl=dsl):
                                y_ps = yps.tile([P, SC], F32, tag="y")
                                for hl in range(2):
                                    nc.tensor.matmul(
                                        y_ps[:],
                                        outT_qc[:, hl, stsl],
                                        wo_sb[:, hl, dsl],
                                        start=(hl == 0),
                                        stop=(hl == 1),
                                    )
                                y_sb = yop.tile([P, SC], F32, tag="ysb")
                                nc.vector.tensor_copy(y_sb[:], y_ps[:])
                                ydst = y[st * P : (st + 1) * P, dsl]
                                if hb == 0:
                                    nc.sync.dma_start(ydst, y_sb[:])
                                else:
                                    nc.gpsimd.dma_start(
                                        ydst, y_sb[:],
                                        accum_op=mybir.AluOpType.add,
                                    )

                            work.append(blk)
                    return work

                pending = deque()
                if causal:
                    for sc in range(NQC):
                        loaded = preloaded if sc == 0 and preloaded else load_chunk(sc)
                        preloaded = None
                        qT_cur = project_chunk(sc, loaded)
                        outT_qc = outqp.tile([P, 2, SC], F32R, tag="outq")
                        attend_chunk(sc, qT_cur, outT_qc, pending)
                        pending.extend(make_out_fillers(hb, sc, outT_qc))
                else:
                    for sc in range(NQC):
                        loaded = preloaded if sc == 0 and preloaded else load_chunk(sc)
                        preloaded = None
                        project_chunk(sc, loaded)
                    for qc in range(NQC):
                        outT_qc = outqp.tile([P, 2, SC], F32R, tag="outq")
                        attend_chunk(qc, qT_full, outT_qc, pending)
                        pending.extend(make_out_fillers(hb, qc, outT_qc))
                while pending:
                    pending.popleft()()

    nc.compile()
    return nc


_NC_CACHE = {}


def _get_nc(causal: bool):
    if causal not in _NC_CACHE:
        _NC_CACHE[causal] = _build_core_kernel(causal)
    return _NC_CACHE[causal]


def _rope_perm_T() -> np.ndarray:
    # rotate_half as a matrix: (P_rh @ q)[d] = -q[d+HD/2] for d < HD/2,
    # q[d-HD/2] otherwise.  Returns P_rh.T for use as matmul lhsT.
    P_rh = np.zeros((HD, HD), dtype=np.float32)
    half = HD // 2
    for i in range(half):
        P_rh[i, half + i] = -1.0
        P_rh[half + i, i] = 1.0
    return np.ascontiguousarray(P_rh.T)


def _is_causal(m: np.ndarray) -> bool:
    tril = np.tril(np.ones((S, S), dtype=bool))
    if not np.all(m[tril] == 0.0):
        return False
    upper = m[~tril]
    return bool(upper.size == 0 or np.all(upper <= -1.0e8))


# module-level: results of the last traced run (for test harnesses)
last_exec_time_ns = None
last_profile_json = None


def kernel(x, cos, sin, mask, wq, wk, wv, wo, _trace=False):
    x = np.asarray(x, dtype=np.float32)
    cos = np.asarray(cos, dtype=np.float32)
    sin = np.asarray(sin, dtype=np.float32)
    mask = np.asarray(mask, dtype=np.float32)
    wq = np.asarray(wq, dtype=np.float32)
    wk = np.asarray(wk, dtype=np.float32)
    wv = np.asarray(wv, dtype=np.float32)
    wo = np.asarray(wo, dtype=np.float32)

    m2d = mask.reshape(S, S)
    causal = _is_causal(m2d)
    nc = _get_nc(causal)

    scale = np.float32(np.sqrt(HD))
    if causal:
        maskT = np.ascontiguousarray((m2d[:SC, :SC] * scale).T)
    else:
        maskT = np.ascontiguousarray((m2d * scale).T)
    cosT = np.ascontiguousarray(cos.T, dtype=np.float32)
    sinT = np.ascontiguousarray(sin.T, dtype=np.float32)
    ptT = _round_f32r(_rope_perm_T())
    ones = np.ones((P, P), dtype=np.float32)

    xT = [_round_f32r(x[b].T) for b in range(B)]

    in_maps = []
    for c in range(N_CORES):
        b = c // (N_CORES // B)
        hg = c % (N_CORES // B)
        rows = slice(hg * HW, (hg + 1) * HW)
        # pack per head-pair: [q_h0 | k_h0 | q_h1 | k_h1 | v_h0 | v_h1]
        packs = []
        for hbp in range(H_LOC // 2):
            cols = []
            for hl in range(2):
                h = hg * H_LOC + hbp * 2 + hl
                cols.append(wq[h * HD : (h + 1) * HD].T)
                cols.append(wk[h * HD : (h + 1) * HD].T)
            for hl in range(2):
                h = hg * H_LOC + hbp * 2 + hl
                cols.append(wv[h * HD : (h + 1) * HD].T)
            packs.append(np.concatenate(cols, axis=1))
        wqkvT = np.stack(packs)
        in_maps.append(
            {
                "xT": xT[b],
                "wqkvT": _round_f32r(wqkvT),
                "woT": _round_f32r(np.ascontiguousarray(wo[:, rows].T)),
                "cosT": cosT,
                "sinT": sinT,
                "PT": ptT,
                "ones": ones,
                "maskT": maskT.astype(np.float32),
            }
        )

    kw = {}
    if _trace:
        kw = dict(trace=True)
    res = run_bass_kernel_spmd(
        nc, in_maps, core_ids=list(range(N_CORES)), **kw
    )
    global last_exec_time_ns, last_profile_json
    last_exec_time_ns = res.exec_time_ns
    last_profile_json = res.profile_json

    out = np.empty((B, S, D), dtype=np.float32)
    gs = N_CORES // B
    for b in range(B):
        acc = res.results[b * gs]["y"].astype(np.float32).copy()
        for g in range(1, gs):
            acc += res.results[b * gs + g]["y"]
        out[b] = acc
    return out



# revision 4
# speedup vs baseline: 1.4721x; 1.4721x over previous
"""Trainium2 8-core kernel for nn_Attention_27530740367526.

Multi-head causal attention (B=2, S=2048, D=2048, H=16, HD=128, fp32) with
RoPE, sharded batch x head-group across 8 NeuronCores: core c handles batch
c//4 and heads [4*(c%4), 4*(c%4)+4).  Each core computes q/k/v projections
(+RoPE), attention for its 4 heads, and the full wo projection contracted
over those heads — a partial [S, D] output.  The host sums the 4 partials
per batch (the row-parallel wo "all-reduce" is a host-side unshard).

v2 design (causal fast path):
- Everything the PE touches is bf16 (1 cycle/row at any moving size, same
  rate as f32r but half the DMA bytes and SBUF footprint).  Scores/softmax
  accumulate in f32 PSUM, so precision loss is only the bf16 rounding of
  x/w/k/v/probs (~0.3% rms, far inside the 2e-2 gate).
- Both head pairs are processed per sequence chunk, so the x chunk is
  loaded ONCE (8 MB bf16 total vs 32 MB f32), with all qkv+wo weights
  resident in SBUF for the whole kernel.
- Transposed land as before: qT/kT are [head_dim, seq], scores come out
  [k, q], softmax denominator is an all-ones-column matmul, PV and wo
  consume natural layouts with zero on-device transposes.
- Causal diagonal trimming: for the diagonal 128-blocks the scores / exp /
  PV / denominator ops are restricted to the valid column range, and the
  mask add shrinks to one persistent [128,128] triangular band.
- wo is one 4-head-contraction pass (no cross-pair DMA accumulate); its
  16 blocks per chunk are drained as PE filler work inside the next
  chunk's attention bubbles, with a 3-deep PSUM rotation in the tail.
- DMA triggers are spread across engines (x on sync, weights on gpsimd,
  wo on scalar, cos/sin on vector) so the initial loads stream in
  parallel and the first projection chain starts within a few us.
"""

import sys

if "/opt/trn_rl_repo" not in sys.path:
    sys.path.insert(0, "/opt/trn_rl_repo")

from collections import deque

import ml_dtypes
import numpy as np

import concourse.bacc as bacc
import concourse.mybir as mybir
import concourse.tile as tile
from concourse.bass_utils import run_bass_kernel_spmd

F32 = mybir.dt.float32
F32R = mybir.dt.float32r
BF16 = mybir.dt.bfloat16
AF = mybir.ActivationFunctionType
NPBF = ml_dtypes.bfloat16

N_HEADS = 16
N_CORES = 8
B, S, D = 2, 2048, 2048
HD = D // N_HEADS
H_LOC = N_HEADS // (N_CORES // B)  # 4 heads per core
HW = H_LOC * HD                    # 512 q/k/v columns per core
SC = 512                           # seq chunk (matmul moving free dim)
P = 128
LOOKAHEAD = 3                      # scores-tile software pipeline depth


def _round_f32r(x: np.ndarray) -> np.ndarray:
    """Host-side fp32 -> float32r rounding (RNE to 11 explicit mantissa
    bits); bit-exact with the device DVE rounding."""
    xi = np.ascontiguousarray(x, dtype=np.float32).view(np.uint32)
    nbits = 12
    lo = np.uint32((1 << nbits) - 1)
    half = np.uint32(1 << (nbits - 1))
    rem = xi & lo
    up = (rem > half) | ((rem == half) & (((xi >> nbits) & 1) == 1))
    r = (xi & ~lo) + np.where(up, np.uint32(1 << nbits), np.uint32(0))
    return r.view(np.float32)


def _build_core_kernel_v2():
    """Causal fast path: bf16 dataflow, single x stream, fused wo."""
    KO = D // P            # 16 contraction subtiles for projections
    NQC = S // SC          # 4 q-chunks
    NSUB = SC // P         # 4 128-blocks per chunk
    NST = S // P           # 16 s-tiles
    NDC = D // SC          # 4 output column chunks
    inv_sqrt_hd = 1.0 / float(np.sqrt(HD))

    nc = bacc.Bacc(None, target_bir_lowering=False)

    xT = nc.dram_tensor("xT", [D, S], BF16, kind="ExternalInput")
    # columns: q h0..h3 | k h0..h3 | v h0..h3 (128 each)
    wqkvT = nc.dram_tensor("wqkvT", [D, 12 * HD], BF16, kind="ExternalInput")
    woT = nc.dram_tensor("woT", [HW, D], BF16, kind="ExternalInput")
    cosT = nc.dram_tensor("cosT", [HD, S], F32, kind="ExternalInput")
    sinT = nc.dram_tensor("sinT", [HD, S], F32, kind="ExternalInput")
    PT = nc.dram_tensor("PT", [HD, HD], BF16, kind="ExternalInput")
    ones = nc.dram_tensor("ones", [P, P], BF16, kind="ExternalInput")
    # one diagonal-band additive mask block, pre-scaled by sqrt(HD)
    tri = nc.dram_tensor("tri", [P, P], F32, kind="ExternalInput")
    y = nc.dram_tensor("y", [S, D], BF16, kind="ExternalOutput")

    xT_r = xT.rearrange("(ko ki) s -> ki ko s", ki=P)
    wqkvT_r = wqkvT.rearrange("(ko ki) c -> ki ko c", ki=P)
    woT_r = woT.rearrange("(h ki) d -> ki h d", ki=P)

    with tile.TileContext(nc) as tc:
        with (
            tc.tile_pool(name="persist", bufs=1) as persist,
            tc.tile_pool(name="wpool", bufs=1) as wpool,
            tc.tile_pool(name="kvq", bufs=1) as kvq,
            tc.tile_pool(name="xa", bufs=2) as xa,
            tc.tile_pool(name="cs", bufs=2) as cspool,
            tc.tile_pool(name="scr", bufs=2) as scr,
            tc.tile_pool(name="exps", bufs=4) as expp,
            tc.tile_pool(name="qtp", bufs=2) as qtp,
            tc.tile_pool(name="yo", bufs=4) as yop,
            tc.tile_pool(name="ps", bufs=3, space="PSUM") as cyc,
            tc.tile_pool(name="ops", bufs=2, space="PSUM") as ops,
            tc.tile_pool(name="dps", bufs=2, space="PSUM") as dps,
            tc.tile_pool(name="yps", bufs=1, space="PSUM") as yps,
        ):
            pt_sb = persist.tile([P, HD], BF16, name="pt_sb")
            nc.scalar.dma_start(pt_sb[:], PT[:])
            ones_sb = persist.tile([P, P], BF16, name="ones_sb")
            nc.scalar.dma_start(ones_sb[:], ones[:])
            tri_sb = persist.tile([P, P], F32, name="tri_sb")
            nc.scalar.dma_start(tri_sb[:], tri[:])

            w_sb = wpool.tile([P, KO, 12 * HD], BF16, name="w_sb")
            wo_sb = wpool.tile([P, H_LOC, D], BF16, name="wo_sb")
            kT_sb = kvq.tile([P, H_LOC, S], BF16, name="kT_sb")
            v_sb = kvq.tile([P, NST, H_LOC * HD], BF16, name="v_sb")
            outT = kvq.tile([P, H_LOC, S], BF16, name="outT")

            def load_x(sc, with_w):
                ssl = slice(sc * SC, (sc + 1) * SC)
                cos_t = cspool.tile([P, SC], F32, tag="cos", name="cos_t")
                sin_t = cspool.tile([P, SC], F32, tag="sin", name="sin_t")
                nc.scalar.dma_start(cos_t[:], cosT[:, ssl])
                nc.scalar.dma_start(sin_t[:], sinT[:, ssl])
                xt = xa.tile([P, KO, SC], BF16, tag="xt", name="xt")
                for ko in range(KO):
                    nc.sync.dma_start(xt[:, ko], xT_r[:, ko, ssl])
                    if with_w:
                        nc.gpsimd.dma_start(w_sb[:, ko, :], wqkvT_r[:, ko, :])
                return xt, cos_t, sin_t

            def project_pair(p, sc, xt, cos_t, sin_t, qT_t):
                """q/k/v projections + RoPE for heads 2p, 2p+1, chunk sc.
                Chains are staggered so each chain's PSUM->SBUF copy and
                rope (ACT/DVE/Pool) overlap the next chain's matmuls."""
                ssl = slice(sc * SC, (sc + 1) * SC)

                def emit_chain(h, t):
                    ps = cyc.tile([P, SC], F32, tag="ps", name="chain_ps")
                    wcol = slice((t * H_LOC + h) * HD, (t * H_LOC + h + 1) * HD)
                    for ko in range(KO):
                        nc.tensor.matmul(
                            ps[:], w_sb[:, ko, wcol], xt[:, ko],
                            start=(ko == 0), stop=(ko == KO - 1),
                        )
                    plain = scr.tile([P, SC], BF16, tag="plain", name="plain")
                    nc.scalar.copy(plain[:], ps[:])
                    return plain

                def emit_rope(h, t, plain):
                    rot = cyc.tile([P, SC], F32, tag="ps", name="rot_ps")
                    nc.tensor.matmul(rot[:], pt_sb[:], plain[:])
                    pc = scr.tile([P, SC], F32, tag="pc", name="pc")
                    nc.gpsimd.tensor_mul(pc[:], plain[:], cos_t[:])
                    tmp2 = scr.tile([P, SC], F32, tag="tmp2", name="tmp2")
                    nc.vector.tensor_mul(tmp2[:], rot[:], sin_t[:])
                    dst = qT_t[:, h, :] if t == 0 else kT_sb[:, h, ssl]
                    nc.vector.tensor_add(dst, pc[:], tmp2[:])

                chains = [(h, t) for h in (2 * p, 2 * p + 1) for t in (0, 1)]
                prev = None
                for c in chains:
                    pl = emit_chain(*c)
                    if prev is not None:
                        emit_rope(prev[0][0], prev[0][1], prev[1])
                    prev = (c, pl)

                for sti in range(NSUB):
                    st = sc * NSUB + sti
                    lsl = slice(sti * P, (sti + 1) * P)
                    vcol = slice(8 * HD + p * 2 * HD, 8 * HD + (p + 1) * 2 * HD)
                    psv = cyc.tile([P, 2 * HD], F32, tag="ps", name="v_ps")
                    for ko in range(KO):
                        nc.tensor.matmul(
                            psv[:], xt[:, ko, lsl], w_sb[:, ko, vcol],
                            start=(ko == 0), stop=(ko == KO - 1),
                        )
                    nc.scalar.copy(
                        v_sb[:, st, p * 2 * HD : (p + 1) * 2 * HD], psv[:]
                    )
                    if prev is not None:
                        emit_rope(prev[0][0], prev[0][1], prev[1])
                        prev = None

            def attend(sc, p, qT_t, fillers):
                """Causal attention for query chunk sc, heads 2p/2p+1
                interleaved per k-block, writing normalized outT slices.
                Diagonal 128-blocks are column-trimmed to the valid range."""
                nkb = (sc + 1) * NSUB
                o_ps, d_ps, qt = {}, {}, {}
                for hl in range(2):
                    qt[hl] = qT_t[:, 2 * p + hl, :]
                    o_ps[hl] = ops.tile([P, SC], F32, tag="o", name=f"o_ps{hl}")
                    d_ps[hl] = dps.tile([P, SC], F32, tag="d", name=f"d_ps{hl}")
                stile = {}

                def emit_scores(kb, hl):
                    h = 2 * p + hl
                    j = kb - sc * NSUB
                    off = j * P if j > 0 else 0
                    t_ = cyc.tile([P, SC], F32, tag="ps", name="score_ps")
                    nc.tensor.matmul(
                        t_[:, off:],
                        kT_sb[:, h, kb * P : (kb + 1) * P],
                        qt[hl][:, off:],
                        skip_group_check=True,
                    )
                    if j >= 0:
                        nc.vector.tensor_add(
                            t_[:, j * P : (j + 1) * P],
                            t_[:, j * P : (j + 1) * P],
                            tri_sb[:],
                        )
                    stile[(kb, hl)] = (t_, off)

                seq = [(kb, hl) for kb in range(nkb) for hl in range(2)]
                for s_ in seq[:LOOKAHEAD]:
                    emit_scores(*s_)
                for i, (kb, hl) in enumerate(seq):
                    h = 2 * p + hl
                    t_, off = stile.pop((kb, hl))
                    e = expp.tile([P, SC], BF16, tag="e", name="e")
                    nc.scalar.activation(
                        e[:, off:], t_[:, off:], AF.Exp, scale=inv_sqrt_hd
                    )
                    nc.tensor.matmul(
                        o_ps[hl][:, off:],
                        v_sb[:, kb, h * HD : (h + 1) * HD],
                        e[:, off:],
                        start=(kb == 0), stop=(kb == nkb - 1),
                        skip_group_check=True,
                    )
                    nc.tensor.matmul(
                        d_ps[hl][:, off:],
                        ones_sb[:],
                        e[:, off:],
                        start=(kb == 0), stop=(kb == nkb - 1),
                        skip_group_check=True,
                    )
                    if i + LOOKAHEAD < len(seq):
                        emit_scores(*seq[i + LOOKAHEAD])
                        if fillers and i % 2 == 1:
                            fillers.popleft()()
                    elif fillers:
                        fillers.popleft()()
                for hl in range(2):
                    h = 2 * p + hl
                    recip = scr.tile([P, SC], F32, tag="recip", name="recip")
                    nc.vector.reciprocal_approx_fast(recip[:], d_ps[hl][:])
                    nc.vector.tensor_mul(
                        outT[:, h, sc * SC : (sc + 1) * SC],
                        o_ps[hl][:], recip[:],
                    )

            def make_wo_blocks(sc, tail=False):
                """One closure per (s-tile, d-chunk) block of the fused wo
                projection for chunk sc: 4 accumulating matmuls over the
                head dim, PSUM->SBUF bf16 copy (alternating DVE/Pool), and
                the output DMA.  Tail blocks rotate through 3 PSUM pools."""
                work = []
                for sti in range(NSUB):
                    st = sc * NSUB + sti
                    for dc in range(NDC):
                        dsl = slice(dc * SC, (dc + 1) * SC)
                        idx = len(work)

                        def blk(st=st, dsl=dsl, idx=idx):
                            if tail:
                                pool, tag = (
                                    (yps, "y"), (ops, "o"), (dps, "d")
                                )[idx % 3]
                            else:
                                pool, tag = yps, "y"
                            y_ps = pool.tile(
                                [P, SC], F32, tag=tag, name="wo_ps"
                            )
                            for h in range(H_LOC):
                                nc.tensor.matmul(
                                    y_ps[:],
                                    outT[:, h, st * P : (st + 1) * P],
                                    wo_sb[:, h, dsl],
                                    start=(h == 0), stop=(h == H_LOC - 1),
                                )
                            y_sb = yop.tile([P, SC], BF16, tag="ysb", name="y_sb")
                            nc.vector.tensor_copy(y_sb[:], y_ps[:])
                            nc.sync.dma_start(y[st * P : (st + 1) * P, dsl], y_sb[:])

                        work.append(blk)
                return work

            pending = deque()
            loaded = load_x(0, with_w=True)
            for h in range(H_LOC):
                nc.scalar.dma_start(wo_sb[:, h], woT_r[:, h, :])
            for sc in range(NQC):
                xt, cos_t, sin_t = loaded
                qT_t = qtp.tile([P, H_LOC, SC], BF16, tag="qT", name="qT_t")
                project_pair(0, sc, xt, cos_t, sin_t, qT_t)
                if sc + 1 < NQC:
                    loaded = load_x(sc + 1, with_w=False)
                attend(sc, 0, qT_t, pending)
                project_pair(1, sc, xt, cos_t, sin_t, qT_t)
                attend(sc, 1, qT_t, pending)
                if sc < NQC - 1:
                    pending.extend(make_wo_blocks(sc))
            while pending:
                pending.popleft()()
            for blk in make_wo_blocks(NQC - 1, tail=True):
                blk()

    nc.compile()
    return nc


def _build_core_kernel_legacy(causal: bool):
    """Baseline f32r kernel, kept as the non-causal fallback."""
    KO = D // P            # 16 contraction subtiles for projections
    NQC = S // SC          # 4 q-chunks
    NSUB = SC // P         # 4 128-blocks per chunk
    NST = S // P           # 16 s-tiles
    NHB = H_LOC // 2       # head pairs
    inv_sqrt_hd = 1.0 / float(np.sqrt(HD))

    nc = bacc.Bacc(None, target_bir_lowering=False)

    xT = nc.dram_tensor("xT", [D, S], F32R, kind="ExternalInput")
    wqkvT = nc.dram_tensor(
        "wqkvT", [H_LOC // 2, D, 6 * HD], F32R, kind="ExternalInput"
    )
    woT = nc.dram_tensor("woT", [HW, D], F32R, kind="ExternalInput")
    cosT = nc.dram_tensor("cosT", [HD, S], F32, kind="ExternalInput")
    sinT = nc.dram_tensor("sinT", [HD, S], F32, kind="ExternalInput")
    PT = nc.dram_tensor("PT", [HD, HD], F32R, kind="ExternalInput")
    ones = nc.dram_tensor("ones", [P, P], F32R, kind="ExternalInput")
    if causal:
        maskT = nc.dram_tensor("maskT", [SC, SC], F32, kind="ExternalInput")
    else:
        maskT = nc.dram_tensor("maskT", [S, S], F32, kind="ExternalInput")
    y = nc.dram_tensor("y", [S, D], F32, kind="ExternalOutput")

    xT_r = xT.rearrange("(ko ki) s -> ki ko s", ki=P)
    wqkvT_r = wqkvT.rearrange("hb (ko ki) c -> hb ki ko c", ki=P)
    woT_r = woT.rearrange("(h ki) d -> ki h d", ki=P)

    with tile.TileContext(nc) as tc:
        with (
            tc.tile_pool(name="persist", bufs=1) as persist,
            tc.tile_pool(name="wpool", bufs=1) as wpool,
            tc.tile_pool(name="kvq", bufs=1) as kvq,
            tc.tile_pool(name="xa", bufs=1) as xa,
            tc.tile_pool(name="cs", bufs=2) as cspool,
            tc.tile_pool(name="scr", bufs=2) as scr,
            tc.tile_pool(name="exps", bufs=4) as expp,
            tc.tile_pool(name="outq", bufs=2) as outqp,
            tc.tile_pool(name="yo", bufs=4) as yop,
            tc.tile_pool(name="gm", bufs=3) as gmp,
            tc.tile_pool(name="ps", bufs=3, space="PSUM") as cyc,
            tc.tile_pool(name="ops", bufs=2, space="PSUM") as ops,
            tc.tile_pool(name="dps", bufs=2, space="PSUM") as dps,
            tc.tile_pool(name="yps", bufs=1, space="PSUM") as yps,
        ):
            pt_sb = persist.tile([P, HD], F32R)
            nc.sync.dma_start(pt_sb[:], PT[:])
            ones_sb = persist.tile([P, P], F32R)
            nc.sync.dma_start(ones_sb[:], ones[:])
            if causal:
                mask_sb = persist.tile([P, NSUB, SC], F32)
                nc.sync.dma_start(
                    mask_sb[:], maskT.rearrange("(j ki) q -> ki j q", ki=P)
                )

            def load_chunk(sc):
                ssl = slice(sc * SC, (sc + 1) * SC)
                xt = xa.tile([P, KO, SC], F32R, tag="xt")
                for ko in range(KO):
                    nc.sync.dma_start(xt[:, ko], xT_r[:, ko, ssl])
                cos_t = cspool.tile([P, SC], F32, tag="cos")
                sin_t = cspool.tile([P, SC], F32, tag="sin")
                nc.sync.dma_start(cos_t[:], cosT[:, ssl])
                nc.sync.dma_start(sin_t[:], sinT[:, ssl])
                return xt, cos_t, sin_t

            preloaded = None
            for hb in range(NHB):
                if hb == 0:
                    preloaded = load_chunk(0)
                w_sb = wpool.tile([P, KO, 6 * HD], F32R, tag="w")
                for ko in range(KO):
                    nc.sync.dma_start(
                        w_sb[:, ko, :], wqkvT_r[hb, :, ko, :]
                    )
                wo_sb = wpool.tile([P, 2, D], F32R, tag="wo")
                for hl in range(2):
                    nc.sync.dma_start(
                        wo_sb[:, hl], woT_r[:, hb * 2 + hl, :]
                    )

                kT_sb = kvq.tile([P, 2, S], F32R, tag="kT")
                v_sb = kvq.tile([P, NST, 2 * HD], F32R, tag="v")
                deferred = not causal
                qT_full = (
                    kvq.tile([P, 2, S], F32R, tag="qT", name="qT_full") if deferred else None
                )

                def project_chunk(sc, loaded):
                    if deferred:
                        qT_dst = qT_full
                    else:
                        qT_dst = outqp.tile([P, 2, SC], F32R, tag="qTc")
                    ssl = slice(sc * SC, (sc + 1) * SC)
                    xt, cos_t, sin_t = loaded

                    for hl in range(2):
                        for t in range(2):  # 0=q, 1=k
                            wcols = slice(
                                (2 * hl + t) * HD, (2 * hl + t + 1) * HD
                            )
                            ps = cyc.tile([P, SC], F32, tag="ps")
                            for ko in range(KO):
                                nc.tensor.matmul(
                                    ps[:],
                                    w_sb[:, ko, wcols],
                                    xt[:, ko],
                                    start=(ko == 0),
                                    stop=(ko == KO - 1),
                                )
                            plain = scr.tile([P, SC], F32R, tag="plain")
                            nc.scalar.copy(plain[:], ps[:])
                            rot = cyc.tile([P, SC], F32, tag="ps")
                            nc.tensor.matmul(rot[:], pt_sb[:], plain[:])
                            if t == 0:
                                dst = (
                                    qT_dst[:, hl, ssl]
                                    if deferred
                                    else qT_dst[:, hl, :]
                                )
                            else:
                                dst = kT_sb[:, hl, ssl]
                            pc = scr.tile([P, SC], F32, tag="pc")
                            nc.gpsimd.tensor_mul(pc[:], plain[:], cos_t[:])
                            tmp2 = scr.tile([P, SC], F32, tag="tmp2")
                            nc.vector.tensor_mul(tmp2[:], rot[:], sin_t[:])
                            nc.vector.tensor_add(dst, pc[:], tmp2[:])

                    for sti in range(NSUB):
                        st = sc * NSUB + sti
                        lsl = slice(sti * P, (sti + 1) * P)
                        psv = cyc.tile([P, 2 * HD], F32, tag="ps")
                        for ko in range(KO):
                            nc.tensor.matmul(
                                psv[:],
                                xt[:, ko, lsl],
                                w_sb[:, ko, 4 * HD : 6 * HD],
                                start=(ko == 0),
                                stop=(ko == KO - 1),
                            )
                        vdst = v_sb[:, st, :]
                        nc.scalar.copy(vdst, psv[:])
                    return qT_dst

                def attend_chunk(qc, qT_cur, outT_qc, fillers):
                    nkb = (qc + 1) * NSUB if causal else NST
                    qt = {}
                    o_ps = {}
                    d_ps = {}
                    for hl in range(2):
                        qt[hl] = (
                            qT_cur[:, hl, qc * SC : (qc + 1) * SC]
                            if deferred
                            else qT_cur[:, hl, :]
                        )
                        o_ps[hl] = ops.tile([P, SC], F32, tag="o", name=f"o_ps{hl}")
                        d_ps[hl] = dps.tile([P, SC], F32, tag="d", name=f"d_ps{hl}")
                    stile = {}

                    def emit_scores(kb, hl):
                        t_ = cyc.tile([P, SC], F32, tag="ps")
                        nc.tensor.matmul(
                            t_[:],
                            kT_sb[:, hl, kb * P : (kb + 1) * P],
                            qt[hl],
                            skip_group_check=True,
                        )
                        if causal:
                            j = kb - qc * NSUB
                            if j >= 0:
                                w_ = P * (j + 1)
                                nc.vector.tensor_add(
                                    t_[:, :w_], t_[:, :w_],
                                    mask_sb[:, j, :w_],
                                )
                        else:
                            if hl == 0:
                                mt = gmp.tile([P, SC], F32, tag="mt")
                                nc.sync.dma_start(
                                    mt[:],
                                    maskT[
                                        kb * P : (kb + 1) * P,
                                        qc * SC : (qc + 1) * SC,
                                    ],
                                )
                                stile[("m", kb)] = mt
                            nc.vector.tensor_add(
                                t_[:], t_[:], stile[("m", kb)][:]
                            )
                        stile[(kb, hl)] = t_

                    seq = [(kb, hl) for kb in range(nkb) for hl in range(2)]
                    for kb, hl in seq[:LOOKAHEAD]:
                        emit_scores(kb, hl)
                    for i, (kb, hl) in enumerate(seq):
                        e = expp.tile([P, SC], F32R, tag="e")
                        nc.scalar.activation(
                            e[:], stile.pop((kb, hl))[:], AF.Exp,
                            scale=inv_sqrt_hd,
                        )
                        nc.tensor.matmul(
                            o_ps[hl][:],
                            v_sb[:, kb, hl * HD : (hl + 1) * HD],
                            e[:],
                            start=(kb == 0),
                            stop=(kb == nkb - 1),
                            skip_group_check=True,
                        )
                        nc.tensor.matmul(
                            d_ps[hl][:],
                            ones_sb[:],
                            e[:],
                            start=(kb == 0),
                            stop=(kb == nkb - 1),
                            skip_group_check=True,
                        )
                        if i + LOOKAHEAD < len(seq):
                            emit_scores(*seq[i + LOOKAHEAD])
                            if fillers and i % 2 == 1:
                                fillers.popleft()()
                        elif fillers:
                            fillers.popleft()()
                    for hl in range(2):
                        recip = scr.tile([P, SC], F32, tag="recip")
                        nc.vector.reciprocal_approx_fast(
                            recip[:], d_ps[hl][:]
                        )
                        nc.vector.tensor_mul(
                            outT_qc[:, hl, :], o_ps[hl][:], recip[:]
                        )

                def make_out_fillers(hb, qc, outT_qc):
                    work = []
                    for sti in range(NSUB):
                        st = qc * NSUB + sti
                        stsl = slice(sti * P, (sti + 1) * P)
                        for dc in range(D // SC):
                            dsl = slice(dc * SC, (dc + 1) * SC)

                            def blk(st=st, stsl=stsl, dsl=dsl):
                                y_ps = yps.tile([P, SC], F32, tag="y")
                                for hl in range(2):
                                    nc.tensor.matmul(
                                        y_ps[:],
                                        outT_qc[:, hl, stsl],
                                        wo_sb[:, hl, dsl],
                                        start=(hl == 0),
                                        stop=(hl == 1),
                                    )
                                y_sb = yop.tile([P, SC], F32, tag="ysb")
                                nc.vector.tensor_copy(y_sb[:], y_ps[:])
                                ydst = y[st * P : (st + 1) * P, dsl]
                                if hb == 0:
                                    nc.sync.dma_start(ydst, y_sb[:])
                                else:
                                    nc.gpsimd.dma_start(
                                        ydst, y_sb[:],
                                        accum_op=mybir.AluOpType.add,
                                    )

                            work.append(blk)
                    return work

                pending = deque()
                if causal:
                    for sc in range(NQC):
                        loaded = preloaded if sc == 0 and preloaded else load_chunk(sc)
                        preloaded = None
                        qT_cur = project_chunk(sc, loaded)
                        outT_qc = outqp.tile([P, 2, SC], F32R, tag="outq")
                        attend_chunk(sc, qT_cur, outT_qc, pending)
                        pending.extend(make_out_fillers(hb, sc, outT_qc))
                else:
                    for sc in range(NQC):
                        loaded = preloaded if sc == 0 and preloaded else load_chunk(sc)
                        preloaded = None
                        project_chunk(sc, loaded)
                    for qc in range(NQC):
                        outT_qc = outqp.tile([P, 2, SC], F32R, tag="outq")
                        attend_chunk(qc, qT_full, outT_qc, pending)
                        pending.extend(make_out_fillers(hb, qc, outT_qc))
                while pending:
                    pending.popleft()()

    nc.compile()
    return nc


_NC_CACHE = {}


def _get_nc(key):
    if key not in _NC_CACHE:
        if key == "v2":
            _NC_CACHE[key] = _build_core_kernel_v2()
        else:
            _NC_CACHE[key] = _build_core_kernel_legacy(causal=False)
    return _NC_CACHE[key]


def _rope_perm_T() -> np.ndarray:
    # rotate_half as a matrix: (P_rh @ q)[d] = -q[d+HD/2] for d < HD/2,
    # q[d-HD/2] otherwise.  Returns P_rh.T for use as matmul lhsT.
    P_rh = np.zeros((HD, HD), dtype=np.float32)
    half = HD // 2
    for i in range(half):
        P_rh[i, half + i] = -1.0
        P_rh[half + i, i] = 1.0
    return np.ascontiguousarray(P_rh.T)


def _is_causal(m: np.ndarray) -> bool:
    tril = np.tril(np.ones((S, S), dtype=bool))
    if not np.all(m[tril] == 0.0):
        return False
    upper = m[~tril]
    return bool(upper.size == 0 or np.all(upper <= -1.0e8))


# module-level: results of the last traced run (for test harnesses)
last_exec_time_ns = None
last_profile_json = None


def _run(nc, in_maps, _trace):
    kw = dict(trace=True) if _trace else {}
    res = run_bass_kernel_spmd(
        nc, in_maps, core_ids=list(range(N_CORES)), **kw
    )
    global last_exec_time_ns, last_profile_json
    last_exec_time_ns = res.exec_time_ns
    last_profile_json = res.profile_json
    return res


def _kernel_v2(x, cos, sin, m2d, wq, wk, wv, wo, _trace):
    scale = np.float32(np.sqrt(HD))
    triT = np.ascontiguousarray((m2d[:P, :P] * scale).T).astype(np.float32)
    cosT = np.ascontiguousarray(cos.T, dtype=np.float32)
    sinT = np.ascontiguousarray(sin.T, dtype=np.float32)
    ptT = _rope_perm_T().astype(NPBF)
    ones = np.ones((P, P), dtype=NPBF)
    xT = [np.ascontiguousarray(x[b].T).astype(NPBF) for b in range(B)]

    in_maps = []
    for c in range(N_CORES):
        b = c // (N_CORES // B)
        hg = c % (N_CORES // B)
        heads = [hg * H_LOC + i for i in range(H_LOC)]
        cols = []
        for w_ in (wq, wk):
            for h in heads:
                cols.append(w_[h * HD : (h + 1) * HD].T)
        for h in heads:
            cols.append(wv[h * HD : (h + 1) * HD].T)
        wqkvT = np.concatenate(cols, axis=1).astype(NPBF)
        rows = slice(hg * HW, (hg + 1) * HW)
        woT = np.ascontiguousarray(wo[:, rows].T).astype(NPBF)
        in_maps.append(
            {
                "xT": xT[b],
                "wqkvT": wqkvT,
                "woT": woT,
                "cosT": cosT,
                "sinT": sinT,
                "PT": ptT,
                "ones": ones,
                "tri": triT,
            }
        )

    res = _run(_get_nc("v2"), in_maps, _trace)

    out = np.empty((B, S, D), dtype=np.float32)
    gs = N_CORES // B
    for b in range(B):
        acc = res.results[b * gs]["y"].astype(np.float32)
        for g in range(1, gs):
            acc = acc + res.results[b * gs + g]["y"].astype(np.float32)
        out[b] = acc
    return out


def _kernel_legacy(x, cos, sin, m2d, wq, wk, wv, wo, _trace):
    scale = np.float32(np.sqrt(HD))
    maskT = np.ascontiguousarray((m2d * scale).T)
    cosT = np.ascontiguousarray(cos.T, dtype=np.float32)
    sinT = np.ascontiguousarray(sin.T, dtype=np.float32)
    ptT = _round_f32r(_rope_perm_T())
    ones = np.ones((P, P), dtype=np.float32)

    xT = [_round_f32r(x[b].T) for b in range(B)]

    in_maps = []
    for c in range(N_CORES):
        b = c // (N_CORES // B)
        hg = c % (N_CORES // B)
        rows = slice(hg * HW, (hg + 1) * HW)
        packs = []
        for hbp in range(H_LOC // 2):
            cols = []
            for hl in range(2):
                h = hg * H_LOC + hbp * 2 + hl
                cols.append(wq[h * HD : (h + 1) * HD].T)
                cols.append(wk[h * HD : (h + 1) * HD].T)
            for hl in range(2):
                h = hg * H_LOC + hbp * 2 + hl
                cols.append(wv[h * HD : (h + 1) * HD].T)
            packs.append(np.concatenate(cols, axis=1))
        wqkvT = np.stack(packs)
        in_maps.append(
            {
                "xT": xT[b],
                "wqkvT": _round_f32r(wqkvT),
                "woT": _round_f32r(np.ascontiguousarray(wo[:, rows].T)),
                "cosT": cosT,
                "sinT": sinT,
                "PT": ptT,
                "ones": ones,
                "maskT": maskT.astype(np.float32),
            }
        )

    res = _run(_get_nc("legacy"), in_maps, _trace)

    out = np.empty((B, S, D), dtype=np.float32)
    gs = N_CORES // B
    for b in range(B):
        acc = res.results[b * gs]["y"].astype(np.float32).copy()
        for g in range(1, gs):
            acc += res.results[b * gs + g]["y"]
        out[b] = acc
    return out


def kernel(x, cos, sin, mask, wq, wk, wv, wo, _trace=False):
    x = np.asarray(x, dtype=np.float32)
    cos = np.asarray(cos, dtype=np.float32)
    sin = np.asarray(sin, dtype=np.float32)
    mask = np.asarray(mask, dtype=np.float32)
    wq = np.asarray(wq, dtype=np.float32)
    wk = np.asarray(wk, dtype=np.float32)
    wv = np.asarray(wv, dtype=np.float32)
    wo = np.asarray(wo, dtype=np.float32)

    m2d = mask.reshape(S, S)
    if _is_causal(m2d):
        return _kernel_v2(x, cos, sin, m2d, wq, wk, wv, wo, _trace)
    return _kernel_legacy(x, cos, sin, m2d, wq, wk, wv, wo, _trace)


# revision 7
# speedup vs baseline: 1.4782x; 1.0042x over previous
"""Trainium2 8-core kernel for nn_Attention_27530740367526.

Multi-head causal attention (B=2, S=2048, D=2048, H=16, HD=128, fp32) with
RoPE, sharded batch x head-group across 8 NeuronCores: core c handles batch
c//4 and heads [4*(c%4), 4*(c%4)+4).  Each core computes q/k/v projections
(+RoPE), attention for its 4 heads, and the full wo projection contracted
over those heads — a partial [S, D] output.  The host sums the 4 partials
per batch (the row-parallel wo "all-reduce" is a host-side unshard).

v2 design (causal fast path):
- Everything the PE touches is bf16 (1 cycle/row at any moving size, same
  rate as f32r but half the DMA bytes and SBUF footprint).  Scores/softmax
  accumulate in f32 PSUM, so precision loss is only the bf16 rounding of
  x/w/k/v/probs (~0.3% rms, far inside the 2e-2 gate).
- Both head pairs are processed per sequence chunk, so the x chunk is
  loaded ONCE (8 MB bf16 total vs 32 MB f32), with all qkv+wo weights
  resident in SBUF for the whole kernel.
- Transposed land as before: qT/kT are [head_dim, seq], scores come out
  [k, q], softmax denominator is an all-ones-column matmul, PV and wo
  consume natural layouts with zero on-device transposes.
- Causal diagonal trimming: for the diagonal 128-blocks the scores / exp /
  PV / denominator ops are restricted to the valid column range, and the
  mask add shrinks to one persistent [128,128] triangular band.
- wo is one 4-head-contraction pass (no cross-pair DMA accumulate); its
  16 blocks per chunk are drained as PE filler work inside the next
  chunk's attention bubbles, with a 3-deep PSUM rotation in the tail.
- DMA triggers are spread across engines (x on sync, weights on gpsimd,
  wo on scalar, cos/sin on vector) so the initial loads stream in
  parallel and the first projection chain starts within a few us.
"""

import sys

if "/opt/trn_rl_repo" not in sys.path:
    sys.path.insert(0, "/opt/trn_rl_repo")

from collections import deque

import ml_dtypes
import numpy as np

import concourse.bacc as bacc
import concourse.mybir as mybir
import concourse.tile as tile
from concourse.bass_utils import run_bass_kernel_spmd

F32 = mybir.dt.float32
F32R = mybir.dt.float32r
BF16 = mybir.dt.bfloat16
AF = mybir.ActivationFunctionType
NPBF = ml_dtypes.bfloat16

N_HEADS = 16
N_CORES = 8
B, S, D = 2, 2048, 2048
HD = D // N_HEADS
H_LOC = N_HEADS // (N_CORES // B)  # 4 heads per core
HW = H_LOC * HD                    # 512 q/k/v columns per core
SC = 512                           # seq chunk (matmul moving free dim)
P = 128
LOOKAHEAD = 3                      # scores-tile software pipeline depth


def _round_f32r(x: np.ndarray) -> np.ndarray:
    """Host-side fp32 -> float32r rounding (RNE to 11 explicit mantissa
    bits); bit-exact with the device DVE rounding."""
    xi = np.ascontiguousarray(x, dtype=np.float32).view(np.uint32)
    nbits = 12
    lo = np.uint32((1 << nbits) - 1)
    half = np.uint32(1 << (nbits - 1))
    rem = xi & lo
    up = (rem > half) | ((rem == half) & (((xi >> nbits) & 1) == 1))
    r = (xi & ~lo) + np.where(up, np.uint32(1 << nbits), np.uint32(0))
    return r.view(np.float32)


def _build_core_kernel_v2():
    """Causal fast path: bf16 dataflow, single x stream, fused wo."""
    KO = D // P            # 16 contraction subtiles for projections
    NQC = S // SC          # 4 q-chunks
    NSUB = SC // P         # 4 128-blocks per chunk
    NST = S // P           # 16 s-tiles
    NDC = D // SC          # 4 output column chunks
    inv_sqrt_hd = 1.0 / float(np.sqrt(HD))

    nc = bacc.Bacc(None, target_bir_lowering=False)

    # x chunk-major: [ki, chunk, ko, s] so one chunk is one 16KB-row DMA
    xTc = nc.dram_tensor("xTc", [P, S // SC, D // P, SC], BF16, kind="ExternalInput")
    # columns: q h0..h3 | k h0..h3 (128 each), v h0..h3 separately
    wqkT = nc.dram_tensor("wqkT", [D, 8 * HD], BF16, kind="ExternalInput")
    wvT = nc.dram_tensor("wvT", [D, 4 * HD], BF16, kind="ExternalInput")
    woT = nc.dram_tensor("woT", [HW, D], BF16, kind="ExternalInput")
    cosT = nc.dram_tensor("cosT", [HD, S], F32, kind="ExternalInput")
    sinT = nc.dram_tensor("sinT", [HD, S], F32, kind="ExternalInput")
    PT = nc.dram_tensor("PT", [HD, HD], BF16, kind="ExternalInput")
    ones = nc.dram_tensor("ones", [P, P], BF16, kind="ExternalInput")
    # one diagonal-band additive mask block, pre-scaled by sqrt(HD)
    tri = nc.dram_tensor("tri", [P, P], F32, kind="ExternalInput")
    y = nc.dram_tensor("y", [S, D], BF16, kind="ExternalOutput")

    wqkT_r = wqkT.rearrange("(ko ki) c -> ki ko c", ki=P)
    wvT_r = wvT.rearrange("(ko ki) c -> ki ko c", ki=P)
    woT_r = woT.rearrange("(h ki) d -> ki h d", ki=P)

    with tile.TileContext(nc) as tc:
        with (
            tc.tile_pool(name="persist", bufs=1) as persist,
            tc.tile_pool(name="wpool", bufs=1) as wpool,
            tc.tile_pool(name="kvq", bufs=1) as kvq,
            tc.tile_pool(name="xa", bufs=2) as xa,
            tc.tile_pool(name="cs", bufs=2) as cspool,
            tc.tile_pool(name="scr", bufs=2) as scr,
            tc.tile_pool(name="exps", bufs=4) as expp,
            tc.tile_pool(name="qtp", bufs=2) as qtp,
            tc.tile_pool(name="yo", bufs=4) as yop,
            tc.tile_pool(name="ps", bufs=3, space="PSUM") as cyc,
            tc.tile_pool(name="ops", bufs=2, space="PSUM") as ops,
            tc.tile_pool(name="dps", bufs=2, space="PSUM") as dps,
            tc.tile_pool(name="yps", bufs=1, space="PSUM") as yps,
        ):
            pt_sb = persist.tile([P, HD], BF16, name="pt_sb")
            nc.scalar.dma_start(pt_sb[:], PT[:])
            ones_sb = persist.tile([P, P], BF16, name="ones_sb")
            nc.scalar.dma_start(ones_sb[:], ones[:])
            tri_sb = persist.tile([P, P], F32, name="tri_sb")
            nc.scalar.dma_start(tri_sb[:], tri[:])

            w_sb = wpool.tile([P, KO, 12 * HD], BF16, name="w_sb")
            wo_sb = wpool.tile([P, H_LOC, D], BF16, name="wo_sb")
            kT_sb = kvq.tile([P, H_LOC, S], BF16, name="kT_sb")
            v_sb = kvq.tile([P, NST, H_LOC * HD], BF16, name="v_sb")
            outT = kvq.tile([P, H_LOC, S], BF16, name="outT")

            def load_x(sc, with_w):
                ssl = slice(sc * SC, (sc + 1) * SC)
                cos_t = cspool.tile([P, SC], F32, tag="cos", name="cos_t")
                sin_t = cspool.tile([P, SC], F32, tag="sin", name="sin_t")
                xt = xa.tile([P, KO, SC], BF16, tag="xt", name="xt")
                if with_w:
                    # warmup: x in 4-ko groups on the sync HWDGE ring, qk
                    # weights in 4-ko groups on the scalar HWDGE ring, and
                    # cos/sin + v weights on the gpsimd SWDGE ring — three
                    # rings stream in parallel and the ko-group-major
                    # first-pair projection consumes groups as they land.
                    nc.gpsimd.dma_start(cos_t[:], cosT[:, ssl])
                    nc.gpsimd.dma_start(sin_t[:], sinT[:, ssl])
                    KG = 4
                    for g in range(KO // KG):
                        gsl = slice(g * KG, (g + 1) * KG)
                        nc.sync.dma_start(xt[:, gsl], xTc[:, sc, gsl])
                        nc.scalar.dma_start(
                            w_sb[:, gsl, : 8 * HD], wqkT_r[:, gsl, :]
                        )
                    for g in range(KO // KG):
                        gsl = slice(g * KG, (g + 1) * KG)
                        nc.gpsimd.dma_start(
                            w_sb[:, gsl, 8 * HD :], wvT_r[:, gsl, :]
                        )
                else:
                    nc.scalar.dma_start(cos_t[:], cosT[:, ssl])
                    nc.scalar.dma_start(sin_t[:], sinT[:, ssl])
                    nc.sync.dma_start(xt[:], xTc[:, sc])
                return xt, cos_t, sin_t

            def project_pair(p, sc, xt, cos_t, sin_t, qT_t, komajor=False):
                """q/k/v projections + RoPE for heads 2p, 2p+1, chunk sc.
                Chains are staggered so each chain's PSUM->SBUF copy and
                rope (ACT/DVE/Pool) overlap the next chain's matmuls."""
                ssl = slice(sc * SC, (sc + 1) * SC)

                def emit_chain(h, t):
                    ps = cyc.tile([P, SC], F32, tag="ps", name="chain_ps")
                    wcol = slice((t * H_LOC + h) * HD, (t * H_LOC + h + 1) * HD)
                    for ko in range(KO):
                        nc.tensor.matmul(
                            ps[:], w_sb[:, ko, wcol], xt[:, ko],
                            start=(ko == 0), stop=(ko == KO - 1),
                        )
                    plain = scr.tile(
                        [P, SC], BF16, tag="plain", name="plain", bufs=3
                    )
                    nc.scalar.copy(plain[:], ps[:])
                    return plain

                def emit_rope(h, t, plain):
                    rot = cyc.tile([P, SC], F32, tag="ps", name="rot_ps")
                    nc.tensor.matmul(rot[:], pt_sb[:], plain[:])
                    pc = scr.tile([P, SC], F32, tag="pc", name="pc")
                    nc.gpsimd.tensor_mul(pc[:], plain[:], cos_t[:])
                    tmp2 = scr.tile([P, SC], F32, tag="tmp2", name="tmp2")
                    nc.vector.tensor_mul(tmp2[:], rot[:], sin_t[:])
                    dst = qT_t[:, h, :] if t == 0 else kT_sb[:, h, ssl]
                    nc.vector.tensor_add(dst, pc[:], tmp2[:])

                chains = [(h, t) for h in (2 * p, 2 * p + 1) for t in (0, 1)]
                prev = None
                if komajor:
                    # chunk 0 is DMA-paced: run the first three chains
                    # ko-group-major so each arriving (x, w) 4-ko group
                    # feeds three matmul chains instead of one.
                    KG = 4
                    pss = []
                    for i in range(3):
                        ps_i = cyc.tile(
                            [P, SC], F32, tag="ps", name=f"km_ps{i}"
                        )
                        pss.append(ps_i)
                    for g in range(KO // KG):
                        for i, (h, t) in enumerate(chains[:3]):
                            wcol = slice(
                                (t * H_LOC + h) * HD,
                                (t * H_LOC + h + 1) * HD,
                            )
                            for ko in range(g * KG, (g + 1) * KG):
                                nc.tensor.matmul(
                                    pss[i][:], w_sb[:, ko, wcol], xt[:, ko],
                                    start=(ko == 0), stop=(ko == KO - 1),
                                    skip_group_check=True,
                                )
                    plains = []
                    for i, (h, t) in enumerate(chains[:3]):
                        pl_i = scr.tile(
                            [P, SC], BF16, tag="plain", name=f"km_pl{i}",
                            bufs=3,
                        )
                        nc.scalar.copy(pl_i[:], pss[i][:])
                        plains.append(pl_i)
                    pl3 = emit_chain(*chains[3])
                    for i, c in enumerate(chains[:3]):
                        emit_rope(c[0], c[1], plains[i])
                    prev = (chains[3], pl3)
                else:
                    for c in chains:
                        pl = emit_chain(*c)
                        if prev is not None:
                            emit_rope(prev[0][0], prev[0][1], prev[1])
                        prev = (c, pl)

                for sti in range(NSUB):
                    st = sc * NSUB + sti
                    lsl = slice(sti * P, (sti + 1) * P)
                    vcol = slice(8 * HD + p * 2 * HD, 8 * HD + (p + 1) * 2 * HD)
                    psv = cyc.tile([P, 2 * HD], F32, tag="ps", name="v_ps")
                    for ko in range(KO):
                        nc.tensor.matmul(
                            psv[:], xt[:, ko, lsl], w_sb[:, ko, vcol],
                            start=(ko == 0), stop=(ko == KO - 1),
                        )
                    nc.scalar.copy(
                        v_sb[:, st, p * 2 * HD : (p + 1) * 2 * HD], psv[:]
                    )
                    if prev is not None:
                        emit_rope(prev[0][0], prev[0][1], prev[1])
                        prev = None

            def attend(sc, p, qT_t, fillers):
                """Causal attention for query chunk sc, heads 2p/2p+1
                interleaved per k-block, writing normalized outT slices.
                Diagonal 128-blocks are column-trimmed to the valid range."""
                nkb = (sc + 1) * NSUB
                o_ps, d_ps, qt = {}, {}, {}
                for hl in range(2):
                    qt[hl] = qT_t[:, 2 * p + hl, :]
                    o_ps[hl] = ops.tile([P, SC], F32, tag="o", name=f"o_ps{hl}")
                    d_ps[hl] = dps.tile([P, SC], F32, tag="d", name=f"d_ps{hl}")
                stile = {}

                def emit_scores(kb, hl):
                    h = 2 * p + hl
                    j = kb - sc * NSUB
                    off = j * P if j > 0 else 0
                    t_ = cyc.tile([P, SC], F32, tag="ps", name="score_ps")
                    nc.tensor.matmul(
                        t_[:, off:],
                        kT_sb[:, h, kb * P : (kb + 1) * P],
                        qt[hl][:, off:],
                        skip_group_check=True,
                    )
                    if j >= 0:
                        nc.vector.tensor_add(
                            t_[:, j * P : (j + 1) * P],
                            t_[:, j * P : (j + 1) * P],
                            tri_sb[:],
                        )
                    stile[(kb, hl)] = (t_, off)

                seq = [(kb, hl) for kb in range(nkb) for hl in range(2)]
                for s_ in seq[:LOOKAHEAD]:
                    emit_scores(*s_)
                for i, (kb, hl) in enumerate(seq):
                    h = 2 * p + hl
                    t_, off = stile.pop((kb, hl))
                    e = expp.tile([P, SC], BF16, tag="e", name="e")
                    nc.scalar.activation(
                        e[:, off:], t_[:, off:], AF.Exp, scale=inv_sqrt_hd
                    )
                    nc.tensor.matmul(
                        o_ps[hl][:, off:],
                        v_sb[:, kb, h * HD : (h + 1) * HD],
                        e[:, off:],
                        start=(kb == 0), stop=(kb == nkb - 1),
                        skip_group_check=True,
                    )
                    nc.tensor.matmul(
                        d_ps[hl][:, off:],
                        ones_sb[:],
                        e[:, off:],
                        start=(kb == 0), stop=(kb == nkb - 1),
                        skip_group_check=True,
                    )
                    if i + LOOKAHEAD < len(seq):
                        emit_scores(*seq[i + LOOKAHEAD])
                        if fillers and i % 2 == 1:
                            fillers.popleft()()
                    elif fillers:
                        fillers.popleft()()
                for hl in range(2):
                    h = 2 * p + hl
                    recip = scr.tile([P, SC], F32, tag="recip", name="recip")
                    nc.vector.reciprocal_approx_fast(recip[:], d_ps[hl][:])
                    nc.vector.tensor_mul(
                        outT[:, h, sc * SC : (sc + 1) * SC],
                        o_ps[hl][:], recip[:],
                    )

            def make_wo_blocks(sc, tail=False):
                """One closure per (s-tile, d-chunk) block of the fused wo
                projection for chunk sc: 4 accumulating matmuls over the
                head dim, PSUM->SBUF bf16 copy (alternating DVE/Pool), and
                the output DMA.  Tail blocks rotate through 3 PSUM pools."""
                work = []
                shared = {}
                for sti in range(NSUB):
                    st = sc * NSUB + sti
                    for dc in range(NDC):
                        dsl = slice(dc * SC, (dc + 1) * SC)
                        idx = len(work)

                        def blk(st=st, dc=dc, dsl=dsl, idx=idx):
                            if tail:
                                pool, tag = (
                                    (yps, "y"), (ops, "o"), (dps, "d")
                                )[idx % 3]
                            else:
                                pool, tag = yps, "y"
                            y_ps = pool.tile(
                                [P, SC], F32, tag=tag, name="wo_ps"
                            )
                            for h in range(H_LOC):
                                nc.tensor.matmul(
                                    y_ps[:],
                                    outT[:, h, st * P : (st + 1) * P],
                                    wo_sb[:, h, dsl],
                                    start=(h == 0), stop=(h == H_LOC - 1),
                                )
                            if dc % 2 == 0:
                                shared["ysb"] = yop.tile(
                                    [P, 2 * SC], BF16, tag="ysb", name="y_sb"
                                )
                            y_sb = shared["ysb"]
                            half = slice((dc % 2) * SC, (dc % 2 + 1) * SC)
                            nc.vector.tensor_copy(y_sb[:, half], y_ps[:])
                            if dc % 2 == 1:
                                nc.sync.dma_start(
                                    y[
                                        st * P : (st + 1) * P,
                                        (dc - 1) * SC : (dc + 1) * SC,
                                    ],
                                    y_sb[:],
                                )

                        work.append(blk)
                return work

            pending = deque()
            loaded = load_x(0, with_w=True)
            for sc in range(NQC):
                xt, cos_t, sin_t = loaded
                qT_t = qtp.tile([P, H_LOC, SC], BF16, tag="qT", name="qT_t")
                project_pair(0, sc, xt, cos_t, sin_t, qT_t, komajor=(sc == 0))
                if sc == 0:
                    # wo is first needed by chunk-1 fillers; stream it on the
                    # (now idle) gpsimd ring behind the v weights.
                    for h in range(H_LOC):
                        nc.gpsimd.dma_start(wo_sb[:, h], woT_r[:, h, :])
                if sc + 1 < NQC:
                    loaded = load_x(sc + 1, with_w=False)
                attend(sc, 0, qT_t, pending)
                project_pair(1, sc, xt, cos_t, sin_t, qT_t)
                attend(sc, 1, qT_t, pending)
                if sc < NQC - 1:
                    pending.extend(make_wo_blocks(sc))
            while pending:
                pending.popleft()()
            for blk in make_wo_blocks(NQC - 1, tail=True):
                blk()

    nc.compile()
    return nc


def _build_core_kernel_legacy(causal: bool):
    """Baseline f32r kernel, kept as the non-causal fallback."""
    KO = D // P            # 16 contraction subtiles for projections
    NQC = S // SC          # 4 q-chunks
    NSUB = SC // P         # 4 128-blocks per chunk
    NST = S // P           # 16 s-tiles
    NHB = H_LOC // 2       # head pairs
    inv_sqrt_hd = 1.0 / float(np.sqrt(HD))

    nc = bacc.Bacc(None, target_bir_lowering=False)

    xT = nc.dram_tensor("xT", [D, S], F32R, kind="ExternalInput")
    wqkvT = nc.dram_tensor(
        "wqkvT", [H_LOC // 2, D, 6 * HD], F32R, kind="ExternalInput"
    )
    woT = nc.dram_tensor("woT", [HW, D], F32R, kind="ExternalInput")
    cosT = nc.dram_tensor("cosT", [HD, S], F32, kind="ExternalInput")
    sinT = nc.dram_tensor("sinT", [HD, S], F32, kind="ExternalInput")
    PT = nc.dram_tensor("PT", [HD, HD], F32R, kind="ExternalInput")
    ones = nc.dram_tensor("ones", [P, P], F32R, kind="ExternalInput")
    if causal:
        maskT = nc.dram_tensor("maskT", [SC, SC], F32, kind="ExternalInput")
    else:
        maskT = nc.dram_tensor("maskT", [S, S], F32, kind="ExternalInput")
    y = nc.dram_tensor("y", [S, D], F32, kind="ExternalOutput")

    xT_r = xT.rearrange("(ko ki) s -> ki ko s", ki=P)
    wqkvT_r = wqkvT.rearrange("hb (ko ki) c -> hb ki ko c", ki=P)
    woT_r = woT.rearrange("(h ki) d -> ki h d", ki=P)

    with tile.TileContext(nc) as tc:
        with (
            tc.tile_pool(name="persist", bufs=1) as persist,
            tc.tile_pool(name="wpool", bufs=1) as wpool,
            tc.tile_pool(name="kvq", bufs=1) as kvq,
            tc.tile_pool(name="xa", bufs=1) as xa,
            tc.tile_pool(name="cs", bufs=2) as cspool,
            tc.tile_pool(name="scr", bufs=2) as scr,
            tc.tile_pool(name="exps", bufs=4) as expp,
            tc.tile_pool(name="outq", bufs=2) as outqp,
            tc.tile_pool(name="yo", bufs=4) as yop,
            tc.tile_pool(name="gm", bufs=3) as gmp,
            tc.tile_pool(name="ps", bufs=3, space="PSUM") as cyc,
            tc.tile_pool(name="ops", bufs=2, space="PSUM") as ops,
            tc.tile_pool(name="dps", bufs=2, space="PSUM") as dps,
            tc.tile_pool(name="yps", bufs=1, space="PSUM") as yps,
        ):
            pt_sb = persist.tile([P, HD], F32R)
            nc.sync.dma_start(pt_sb[:], PT[:])
            ones_sb = persist.tile([P, P], F32R)
            nc.sync.dma_start(ones_sb[:], ones[:])
            if causal:
                mask_sb = persist.tile([P, NSUB, SC], F32)
                nc.sync.dma_start(
                    mask_sb[:], maskT.rearrange("(j ki) q -> ki j q", ki=P)
                )

            def load_chunk(sc):
                ssl = slice(sc * SC, (sc + 1) * SC)
                xt = xa.tile([P, KO, SC], F32R, tag="xt")
                for ko in range(KO):
                    nc.sync.dma_start(xt[:, ko], xT_r[:, ko, ssl])
                cos_t = cspool.tile([P, SC], F32, tag="cos")
                sin_t = cspool.tile([P, SC], F32, tag="sin")
                nc.sync.dma_start(cos_t[:], cosT[:, ssl])
                nc.sync.dma_start(sin_t[:], sinT[:, ssl])
                return xt, cos_t, sin_t

            preloaded = None
            for hb in range(NHB):
                if hb == 0:
                    preloaded = load_chunk(0)
                w_sb = wpool.tile([P, KO, 6 * HD], F32R, tag="w")
                for ko in range(KO):
                    nc.sync.dma_start(
                        w_sb[:, ko, :], wqkvT_r[hb, :, ko, :]
                    )
                wo_sb = wpool.tile([P, 2, D], F32R, tag="wo")
                for hl in range(2):
                    nc.sync.dma_start(
                        wo_sb[:, hl], woT_r[:, hb * 2 + hl, :]
                    )

                kT_sb = kvq.tile([P, 2, S], F32R, tag="kT")
                v_sb = kvq.tile([P, NST, 2 * HD], F32R, tag="v")
                deferred = not causal
                qT_full = (
                    kvq.tile([P, 2, S], F32R, tag="qT", name="qT_full") if deferred else None
                )

                def project_chunk(sc, loaded):
                    if deferred:
                        qT_dst = qT_full
                    else:
                        qT_dst = outqp.tile([P, 2, SC], F32R, tag="qTc")
                    ssl = slice(sc * SC, (sc + 1) * SC)
                    xt, cos_t, sin_t = loaded

                    for hl in range(2):
                        for t in range(2):  # 0=q, 1=k
                            wcols = slice(
                                (2 * hl + t) * HD, (2 * hl + t + 1) * HD
                            )
                            ps = cyc.tile([P, SC], F32, tag="ps")
                            for ko in range(KO):
                                nc.tensor.matmul(
                                    ps[:],
                                    w_sb[:, ko, wcols],
                                    xt[:, ko],
                                    start=(ko == 0),
                                    stop=(ko == KO - 1),
                                )
                            plain = scr.tile([P, SC], F32R, tag="plain")
                            nc.scalar.copy(plain[:], ps[:])
                            rot = cyc.tile([P, SC], F32, tag="ps")
                            nc.tensor.matmul(rot[:], pt_sb[:], plain[:])
                            if t == 0:
                                dst = (
                                    qT_dst[:, hl, ssl]
                                    if deferred
                                    else qT_dst[:, hl, :]
                                )
                            else:
                                dst = kT_sb[:, hl, ssl]
                            pc = scr.tile([P, SC], F32, tag="pc")
                            nc.gpsimd.tensor_mul(pc[:], plain[:], cos_t[:])
                            tmp2 = scr.tile([P, SC], F32, tag="tmp2")
                            nc.vector.tensor_mul(tmp2[:], rot[:], sin_t[:])
                            nc.vector.tensor_add(dst, pc[:], tmp2[:])

                    for sti in range(NSUB):
                        st = sc * NSUB + sti
                        lsl = slice(sti * P, (sti + 1) * P)
                        psv = cyc.tile([P, 2 * HD], F32, tag="ps")
                        for ko in range(KO):
                            nc.tensor.matmul(
                                psv[:],
                                xt[:, ko, lsl],
                                w_sb[:, ko, 4 * HD : 6 * HD],
                                start=(ko == 0),
                                stop=(ko == KO - 1),
                            )
                        vdst = v_sb[:, st, :]
                        nc.scalar.copy(vdst, psv[:])
                    return qT_dst

                def attend_chunk(qc, qT_cur, outT_qc, fillers):
                    nkb = (qc + 1) * NSUB if causal else NST
                    qt = {}
                    o_ps = {}
                    d_ps = {}
                    for hl in range(2):
                        qt[hl] = (
                            qT_cur[:, hl, qc * SC : (qc + 1) * SC]
                            if deferred
                            else qT_cur[:, hl, :]
                        )
                        o_ps[hl] = ops.tile([P, SC], F32, tag="o", name=f"o_ps{hl}")
                        d_ps[hl] = dps.tile([P, SC], F32, tag="d", name=f"d_ps{hl}")
                    stile = {}

                    def emit_scores(kb, hl):
                        t_ = cyc.tile([P, SC], F32, tag="ps")
                        nc.tensor.matmul(
                            t_[:],
                            kT_sb[:, hl, kb * P : (kb + 1) * P],
                            qt[hl],
                            skip_group_check=True,
                        )
                        if causal:
                            j = kb - qc * NSUB
                            if j >= 0:
                                w_ = P * (j + 1)
                                nc.vector.tensor_add(
                                    t_[:, :w_], t_[:, :w_],
                                    mask_sb[:, j, :w_],
                                )
                        else:
                            if hl == 0:
                                mt = gmp.tile([P, SC], F32, tag="mt")
                                nc.sync.dma_start(
                                    mt[:],
                                    maskT[
                                        kb * P : (kb + 1) * P,
                                        qc * SC : (qc + 1) * SC,
                                    ],
                                )
                                stile[("m", kb)] = mt
                            nc.vector.tensor_add(
                                t_[:], t_[:], stile[("m", kb)][:]
                            )
                        stile[(kb, hl)] = t_

                    seq = [(kb, hl) for kb in range(nkb) for hl in range(2)]
                    for kb, hl in seq[:LOOKAHEAD]:
                        emit_scores(kb, hl)
                    for i, (kb, hl) in enumerate(seq):
                        e = expp.tile([P, SC], F32R, tag="e")
                        nc.scalar.activation(
                            e[:], stile.pop((kb, hl))[:], AF.Exp,
                            scale=inv_sqrt_hd,
                        )
                        nc.tensor.matmul(
                            o_ps[hl][:],
                            v_sb[:, kb, hl * HD : (hl + 1) * HD],
                            e[:],
                            start=(kb == 0),
                            stop=(kb == nkb - 1),
                            skip_group_check=True,
                        )
                        nc.tensor.matmul(
                            d_ps[hl][:],
                            ones_sb[:],
                            e[:],
                            start=(kb == 0),
                            stop=(kb == nkb - 1),
                            skip_group_check=True,
                        )
                        if i + LOOKAHEAD < len(seq):
                            emit_scores(*seq[i + LOOKAHEAD])
                            if fillers and i % 2 == 1:
                                fillers.popleft()()
                        elif fillers:
                            fillers.popleft()()
                    for hl in range(2):
                        recip = scr.tile([P, SC], F32, tag="recip")
                        nc.vector.reciprocal_approx_fast(
                            recip[:], d_ps[hl][:]
                        )
                        nc.vector.tensor_mul(
                            outT_qc[:, hl, :], o_ps[hl][:], recip[:]
                        )

                def make_out_fillers(hb, qc, outT_qc):
                    work = []
                    for sti in range(NSUB):
                        st = qc * NSUB + sti
                        stsl = slice(sti * P, (sti + 1) * P)
                        for dc in range(D // SC):
                            dsl = slice(dc * SC, (dc + 1) * SC)

                            def blk(st=st, stsl=stsl, dsl=dsl):
                                y_ps = yps.tile([P, SC], F32, tag="y")
                                for hl in range(2):
                                    nc.tensor.matmul(
                                        y_ps[:],
                                        outT_qc[:, hl, stsl],
                                        wo_sb[:, hl, dsl],
                                        start=(hl == 0),
                                        stop=(hl == 1),
                                    )
                                y_sb = yop.tile([P, SC], F32, tag="ysb")
                                nc.vector.tensor_copy(y_sb[:], y_ps[:])
                                ydst = y[st * P : (st + 1) * P, dsl]
                                if hb == 0:
                                    nc.sync.dma_start(ydst, y_sb[:])
                                else:
                                    nc.gpsimd.dma_start(
                                        ydst, y_sb[:],
                                        accum_op=mybir.AluOpType.add,
                                    )

                            work.append(blk)
                    return work

                pending = deque()
                if causal:
                    for sc in range(NQC):
                        loaded = preloaded if sc == 0 and preloaded else load_chunk(sc)
                        preloaded = None
                        qT_cur = project_chunk(sc, loaded)
                        outT_qc = outqp.tile([P, 2, SC], F32R, tag="outq")
                        attend_chunk(sc, qT_cur, outT_qc, pending)
                        pending.extend(make_out_fillers(hb, sc, outT_qc))
                else:
                    for sc in range(NQC):
                        loaded = preloaded if sc == 0 and preloaded else load_chunk(sc)
                        preloaded = None
                        project_chunk(sc, loaded)
                    for qc in range(NQC):
                        outT_qc = outqp.tile([P, 2, SC], F32R, tag="outq")
                        attend_chunk(qc, qT_full, outT_qc, pending)
                        pending.extend(make_out_fillers(hb, qc, outT_qc))
                while pending:
                    pending.popleft()()

    nc.compile()
    return nc


_NC_CACHE = {}


def _get_nc(key):
    if key not in _NC_CACHE:
        if key == "v2":
            _NC_CACHE[key] = _build_core_kernel_v2()
        else:
            _NC_CACHE[key] = _build_core_kernel_legacy(causal=False)
    return _NC_CACHE[key]


def _rope_perm_T() -> np.ndarray:
    # rotate_half as a matrix: (P_rh @ q)[d] = -q[d+HD/2] for d < HD/2,
    # q[d-HD/2] otherwise.  Returns P_rh.T for use as matmul lhsT.
    P_rh = np.zeros((HD, HD), dtype=np.float32)
    half = HD // 2
    for i in range(half):
        P_rh[i, half + i] = -1.0
        P_rh[half + i, i] = 1.0
    return np.ascontiguousarray(P_rh.T)


def _is_causal(m: np.ndarray) -> bool:
    tril = np.tril(np.ones((S, S), dtype=bool))
    if not np.all(m[tril] == 0.0):
        return False
    upper = m[~tril]
    return bool(upper.size == 0 or np.all(upper <= -1.0e8))


# module-level: results of the last traced run (for test harnesses)
last_exec_time_ns = None
last_profile_json = None


def _run(nc, in_maps, _trace):
    kw = dict(trace=True) if _trace else {}
    res = run_bass_kernel_spmd(
        nc, in_maps, core_ids=list(range(N_CORES)), **kw
    )
    global last_exec_time_ns, last_profile_json
    last_exec_time_ns = res.exec_time_ns
    last_profile_json = res.profile_json
    return res


def _kernel_v2(x, cos, sin, m2d, wq, wk, wv, wo, _trace):
    scale = np.float32(np.sqrt(HD))
    triT = np.ascontiguousarray((m2d[:P, :P] * scale).T).astype(np.float32)
    cosT = np.ascontiguousarray(cos.T, dtype=np.float32)
    sinT = np.ascontiguousarray(sin.T, dtype=np.float32)
    ptT = _rope_perm_T().astype(NPBF)
    ones = np.ones((P, P), dtype=NPBF)
    # chunk-major x: [ki, chunk, ko, s] = xT[ko*128+ki, chunk*512+s]
    xTc = [
        np.ascontiguousarray(
            x[b].T.reshape(D // P, P, S // SC, SC).transpose(1, 2, 0, 3)
        ).astype(NPBF)
        for b in range(B)
    ]

    in_maps = []
    for c in range(N_CORES):
        b = c // (N_CORES // B)
        hg = c % (N_CORES // B)
        heads = [hg * H_LOC + i for i in range(H_LOC)]
        cols = []
        for w_ in (wq, wk):
            for h in heads:
                cols.append(w_[h * HD : (h + 1) * HD].T)
        wqkT = np.concatenate(cols, axis=1).astype(NPBF)
        wvT = np.concatenate(
            [wv[h * HD : (h + 1) * HD].T for h in heads], axis=1
        ).astype(NPBF)
        rows = slice(hg * HW, (hg + 1) * HW)
        woT = np.ascontiguousarray(wo[:, rows].T).astype(NPBF)
        in_maps.append(
            {
                "xTc": xTc[b],
                "wqkT": wqkT,
                "wvT": wvT,
                "woT": woT,
                "cosT": cosT,
                "sinT": sinT,
                "PT": ptT,
                "ones": ones,
                "tri": triT,
            }
        )

    res = _run(_get_nc("v2"), in_maps, _trace)

    out = np.empty((B, S, D), dtype=np.float32)
    gs = N_CORES // B
    for b in range(B):
        acc = res.results[b * gs]["y"].astype(np.float32)
        for g in range(1, gs):
            acc = acc + res.results[b * gs + g]["y"].astype(np.float32)
        out[b] = acc
    return out


def _kernel_legacy(x, cos, sin, m2d, wq, wk, wv, wo, _trace):
    scale = np.float32(np.sqrt(HD))
    maskT = np.ascontiguousarray((m2d * scale).T)
    cosT = np.ascontiguousarray(cos.T, dtype=np.float32)
    sinT = np.ascontiguousarray(sin.T, dtype=np.float32)
    ptT = _round_f32r(_rope_perm_T())
    ones = np.ones((P, P), dtype=np.float32)

    xT = [_round_f32r(x[b].T) for b in range(B)]

    in_maps = []
    for c in range(N_CORES):
        b = c // (N_CORES // B)
        hg = c % (N_CORES // B)
        rows = slice(hg * HW, (hg + 1) * HW)
        packs = []
        for hbp in range(H_LOC // 2):
            cols = []
            for hl in range(2):
                h = hg * H_LOC + hbp * 2 + hl
                cols.append(wq[h * HD : (h + 1) * HD].T)
                cols.append(wk[h * HD : (h + 1) * HD].T)
            for hl in range(2):
                h = hg * H_LOC + hbp * 2 + hl
                cols.append(wv[h * HD : (h + 1) * HD].T)
            packs.append(np.concatenate(cols, axis=1))
        wqkvT = np.stack(packs)
        in_maps.append(
            {
                "xT": xT[b],
                "wqkvT": _round_f32r(wqkvT),
                "woT": _round_f32r(np.ascontiguousarray(wo[:, rows].T)),
                "cosT": cosT,
                "sinT": sinT,
                "PT": ptT,
                "ones": ones,
                "maskT": maskT.astype(np.float32),
            }
        )

    res = _run(_get_nc("legacy"), in_maps, _trace)

    out = np.empty((B, S, D), dtype=np.float32)
    gs = N_CORES // B
    for b in range(B):
        acc = res.results[b * gs]["y"].astype(np.float32).copy()
        for g in range(1, gs):
            acc += res.results[b * gs + g]["y"]
        out[b] = acc
    return out


def kernel(x, cos, sin, mask, wq, wk, wv, wo, _trace=False):
    x = np.asarray(x, dtype=np.float32)
    cos = np.asarray(cos, dtype=np.float32)
    sin = np.asarray(sin, dtype=np.float32)
    mask = np.asarray(mask, dtype=np.float32)
    wq = np.asarray(wq, dtype=np.float32)
    wk = np.asarray(wk, dtype=np.float32)
    wv = np.asarray(wv, dtype=np.float32)
    wo = np.asarray(wo, dtype=np.float32)

    m2d = mask.reshape(S, S)
    if _is_causal(m2d):
        return _kernel_v2(x, cos, sin, m2d, wq, wk, wv, wo, _trace)
    return _kernel_legacy(x, cos, sin, m2d, wq, wk, wv, wo, _trace)


# revision 9
# speedup vs baseline: 1.5236x; 1.0307x over previous
"""Trainium2 8-core kernel for nn_Attention_27530740367526.

Multi-head causal attention (B=2, S=2048, D=2048, H=16, HD=128, fp32) with
RoPE, sharded batch x head-group across 8 NeuronCores: core c handles batch
c//4 and heads [4*(c%4), 4*(c%4)+4).  Each core computes q/k/v projections
(+RoPE), attention for its 4 heads, and the full wo projection contracted
over those heads — a partial [S, D] output.  The host sums the 4 partials
per batch (the row-parallel wo "all-reduce" is a host-side unshard).

v2 design (causal fast path):
- Everything the PE touches is bf16 (1 cycle/row at any moving size, same
  rate as f32r but half the DMA bytes and SBUF footprint).  Scores/softmax
  accumulate in f32 PSUM, so precision loss is only the bf16 rounding of
  x/w/k/v/probs (~0.3% rms, far inside the 2e-2 gate).
- Both head pairs are processed per sequence chunk, so the x chunk is
  loaded ONCE (8 MB bf16 total vs 32 MB f32), with all qkv+wo weights
  resident in SBUF for the whole kernel.
- Transposed land as before: qT/kT are [head_dim, seq], scores come out
  [k, q], softmax denominator is an all-ones-column matmul, PV and wo
  consume natural layouts with zero on-device transposes.
- Causal diagonal trimming: for the diagonal 128-blocks the scores / exp /
  PV / denominator ops are restricted to the valid column range, and the
  mask add shrinks to one persistent [128,128] triangular band.
- wo is one 4-head-contraction pass (no cross-pair DMA accumulate); its
  16 blocks per chunk are drained as PE filler work inside the next
  chunk's attention bubbles, with a 3-deep PSUM rotation in the tail.
- DMA triggers are spread across engines (x on sync, weights on gpsimd,
  wo on scalar, cos/sin on vector) so the initial loads stream in
  parallel and the first projection chain starts within a few us.
"""

import sys

if "/opt/trn_rl_repo" not in sys.path:
    sys.path.insert(0, "/opt/trn_rl_repo")

from collections import deque

import ml_dtypes
import numpy as np

import concourse.bacc as bacc
import concourse.mybir as mybir
import concourse.tile as tile
from concourse.bass_utils import run_bass_kernel_spmd

F32 = mybir.dt.float32
F32R = mybir.dt.float32r
BF16 = mybir.dt.bfloat16
AF = mybir.ActivationFunctionType
NPBF = ml_dtypes.bfloat16

N_HEADS = 16
N_CORES = 8
B, S, D = 2, 2048, 2048
HD = D // N_HEADS
H_LOC = N_HEADS // (N_CORES // B)  # 4 heads per core
HW = H_LOC * HD                    # 512 q/k/v columns per core
SC = 512                           # seq chunk (matmul moving free dim)
P = 128
LOOKAHEAD = 3                      # scores-tile software pipeline depth


def _round_f32r(x: np.ndarray) -> np.ndarray:
    """Host-side fp32 -> float32r rounding (RNE to 11 explicit mantissa
    bits); bit-exact with the device DVE rounding."""
    xi = np.ascontiguousarray(x, dtype=np.float32).view(np.uint32)
    nbits = 12
    lo = np.uint32((1 << nbits) - 1)
    half = np.uint32(1 << (nbits - 1))
    rem = xi & lo
    up = (rem > half) | ((rem == half) & (((xi >> nbits) & 1) == 1))
    r = (xi & ~lo) + np.where(up, np.uint32(1 << nbits), np.uint32(0))
    return r.view(np.float32)


def _build_core_kernel_v2():
    """Causal fast path: bf16 dataflow, single x stream, fused wo."""
    KO = D // P            # 16 contraction subtiles for projections
    NQC = S // SC          # 4 q-chunks
    NSUB = SC // P         # 4 128-blocks per chunk
    NST = S // P           # 16 s-tiles
    NDC = D // SC          # 4 output column chunks
    inv_sqrt_hd = 1.0 / float(np.sqrt(HD))

    nc = bacc.Bacc(None, target_bir_lowering=False)

    # x chunk-major: [ki, chunk, ko, s] so one chunk is one 16KB-row DMA
    xTc = nc.dram_tensor("xTc", [P, S // SC, D // P, SC], BF16, kind="ExternalInput")
    # weights partition-major: [ki, ko, cols] so group DMAs move 8KB/4KB
    # contiguous runs per partition (descriptor-count-optimal)
    wqkG = nc.dram_tensor("wqkG", [P, D // P, 8 * HD], BF16, kind="ExternalInput")
    wvG = nc.dram_tensor("wvG", [P, D // P, 4 * HD], BF16, kind="ExternalInput")
    woG = nc.dram_tensor("woG", [P, H_LOC, D], BF16, kind="ExternalInput")
    cosT = nc.dram_tensor("cosT", [HD, S], F32, kind="ExternalInput")
    sinT = nc.dram_tensor("sinT", [HD, S], F32, kind="ExternalInput")
    PT = nc.dram_tensor("PT", [HD, HD], BF16, kind="ExternalInput")
    ones = nc.dram_tensor("ones", [P, P], BF16, kind="ExternalInput")
    # one diagonal-band additive mask block, pre-scaled by sqrt(HD)
    tri = nc.dram_tensor("tri", [P, P], F32, kind="ExternalInput")
    y = nc.dram_tensor("y", [S, D], BF16, kind="ExternalOutput")


    with tile.TileContext(nc) as tc:
        with (
            tc.tile_pool(name="persist", bufs=1) as persist,
            tc.tile_pool(name="wpool", bufs=1) as wpool,
            tc.tile_pool(name="kvq", bufs=1) as kvq,
            tc.tile_pool(name="xa", bufs=2) as xa,
            tc.tile_pool(name="cs", bufs=2) as cspool,
            tc.tile_pool(name="scr", bufs=2) as scr,
            tc.tile_pool(name="exps", bufs=4) as expp,
            tc.tile_pool(name="qtp", bufs=2) as qtp,
            tc.tile_pool(name="yo", bufs=4) as yop,
            tc.tile_pool(name="ps", bufs=3, space="PSUM") as cyc,
            tc.tile_pool(name="ops", bufs=2, space="PSUM") as ops,
            tc.tile_pool(name="dps", bufs=2, space="PSUM") as dps,
            tc.tile_pool(name="yps", bufs=1, space="PSUM") as yps,
        ):
            pt_sb = persist.tile([P, HD], BF16, name="pt_sb")
            nc.gpsimd.dma_start(pt_sb[:], PT[:])
            ones_sb = persist.tile([P, P], BF16, name="ones_sb")
            nc.gpsimd.dma_start(ones_sb[:], ones[:])
            tri_sb = persist.tile([P, P], F32, name="tri_sb")
            nc.gpsimd.dma_start(tri_sb[:], tri[:])

            wqk_sb = wpool.tile([P, KO, 8 * HD], BF16, name="wqk_sb")
            wv_sb = wpool.tile([P, KO, 4 * HD], BF16, name="wv_sb")
            wo_sb = wpool.tile([P, H_LOC, D], BF16, name="wo_sb")
            kT_sb = kvq.tile([P, H_LOC, S], BF16, name="kT_sb")
            v_sb = kvq.tile([P, NST, H_LOC * HD], BF16, name="v_sb")
            outT = kvq.tile([P, H_LOC, S], BF16, name="outT")

            def load_x(sc, with_w):
                ssl = slice(sc * SC, (sc + 1) * SC)
                cos_t = cspool.tile([P, SC], F32, tag="cos", name="cos_t")
                sin_t = cspool.tile([P, SC], F32, tag="sin", name="sin_t")
                xt = xa.tile([P, KO, SC], BF16, tag="xt", name="xt")
                if with_w:
                    # warmup: x in 4-ko groups on the sync HWDGE ring, qk
                    # weights in 4-ko groups on the scalar HWDGE ring, and
                    # consts + cos/sin + v weights on the gpsimd SWDGE
                    # ring — three rings stream in parallel, all with
                    # multi-KB contiguous runs, and the ko-group-major
                    # first-pair projection consumes groups as they land.
                    nc.gpsimd.dma_start(cos_t[:], cosT[:, ssl])
                    nc.gpsimd.dma_start(sin_t[:], sinT[:, ssl])
                    KG = 4
                    for g in range(KO // KG):
                        gsl = slice(g * KG, (g + 1) * KG)
                        nc.sync.dma_start(xt[:, gsl], xTc[:, sc, gsl])
                        nc.scalar.dma_start(wqk_sb[:, gsl], wqkG[:, gsl])
                    for g in range(KO // KG):
                        gsl = slice(g * KG, (g + 1) * KG)
                        nc.gpsimd.dma_start(wv_sb[:, gsl], wvG[:, gsl])
                else:
                    nc.scalar.dma_start(cos_t[:], cosT[:, ssl])
                    nc.scalar.dma_start(sin_t[:], sinT[:, ssl])
                    nc.sync.dma_start(xt[:], xTc[:, sc])
                return xt, cos_t, sin_t

            def project_pair(p, sc, xt, cos_t, sin_t, qT_t, komajor=False):
                """q/k/v projections + RoPE for heads 2p, 2p+1, chunk sc.
                Chains are staggered so each chain's PSUM->SBUF copy and
                rope (ACT/DVE/Pool) overlap the next chain's matmuls."""
                ssl = slice(sc * SC, (sc + 1) * SC)

                def emit_chain(h, t):
                    ps = cyc.tile([P, SC], F32, tag="ps", name="chain_ps")
                    wcol = slice((t * H_LOC + h) * HD, (t * H_LOC + h + 1) * HD)
                    for ko in range(KO):
                        nc.tensor.matmul(
                            ps[:], wqk_sb[:, ko, wcol], xt[:, ko],
                            start=(ko == 0), stop=(ko == KO - 1),
                        )
                    plain = scr.tile(
                        [P, SC], BF16, tag="plain", name="plain", bufs=3
                    )
                    nc.scalar.copy(plain[:], ps[:])
                    return plain

                def emit_rope(h, t, plain):
                    rot = cyc.tile([P, SC], F32, tag="ps", name="rot_ps")
                    nc.tensor.matmul(rot[:], pt_sb[:], plain[:])
                    pc = scr.tile([P, SC], F32, tag="pc", name="pc")
                    nc.gpsimd.tensor_mul(pc[:], plain[:], cos_t[:])
                    tmp2 = scr.tile([P, SC], F32, tag="tmp2", name="tmp2")
                    nc.vector.tensor_mul(tmp2[:], rot[:], sin_t[:])
                    dst = qT_t[:, h, :] if t == 0 else kT_sb[:, h, ssl]
                    nc.vector.tensor_add(dst, pc[:], tmp2[:])

                chains = [(h, t) for h in (2 * p, 2 * p + 1) for t in (0, 1)]
                prev = None
                if komajor:
                    # chunk 0 is DMA-paced: run the first three chains
                    # ko-group-major so each arriving (x, w) 4-ko group
                    # feeds three matmul chains instead of one.
                    KG = 4
                    pss = []
                    for i in range(3):
                        ps_i = cyc.tile(
                            [P, SC], F32, tag="ps", name=f"km_ps{i}"
                        )
                        pss.append(ps_i)
                    for g in range(KO // KG):
                        for i, (h, t) in enumerate(chains[:3]):
                            wcol = slice(
                                (t * H_LOC + h) * HD,
                                (t * H_LOC + h + 1) * HD,
                            )
                            for ko in range(g * KG, (g + 1) * KG):
                                nc.tensor.matmul(
                                    pss[i][:], wqk_sb[:, ko, wcol], xt[:, ko],
                                    start=(ko == 0), stop=(ko == KO - 1),
                                    skip_group_check=True,
                                )
                    plains = []
                    for i, (h, t) in enumerate(chains[:3]):
                        pl_i = scr.tile(
                            [P, SC], BF16, tag="plain", name=f"km_pl{i}",
                            bufs=3,
                        )
                        nc.scalar.copy(pl_i[:], pss[i][:])
                        plains.append(pl_i)
                    pl3 = emit_chain(*chains[3])
                    for i, c in enumerate(chains[:3]):
                        emit_rope(c[0], c[1], plains[i])
                    prev = (chains[3], pl3)
                else:
                    for c in chains:
                        pl = emit_chain(*c)
                        if prev is not None:
                            emit_rope(prev[0][0], prev[0][1], prev[1])
                        prev = (c, pl)

                for sti in range(NSUB):
                    st = sc * NSUB + sti
                    lsl = slice(sti * P, (sti + 1) * P)
                    vcol = slice(p * 2 * HD, (p + 1) * 2 * HD)
                    psv = cyc.tile([P, 2 * HD], F32, tag="ps", name="v_ps")
                    for ko in range(KO):
                        nc.tensor.matmul(
                            psv[:], xt[:, ko, lsl], wv_sb[:, ko, vcol],
                            start=(ko == 0), stop=(ko == KO - 1),
                        )
                    nc.scalar.copy(
                        v_sb[:, st, p * 2 * HD : (p + 1) * 2 * HD], psv[:]
                    )
                    if prev is not None:
                        emit_rope(prev[0][0], prev[0][1], prev[1])
                        prev = None

            def attend(sc, p, qT_t, fillers):
                """Causal attention for query chunk sc, heads 2p/2p+1
                interleaved per k-block, writing normalized outT slices.
                Diagonal 128-blocks are column-trimmed to the valid range."""
                nkb = (sc + 1) * NSUB
                o_ps, d_ps, qt = {}, {}, {}
                for hl in range(2):
                    qt[hl] = qT_t[:, 2 * p + hl, :]
                    o_ps[hl] = ops.tile([P, SC], F32, tag="o", name=f"o_ps{hl}")
                    d_ps[hl] = dps.tile([P, SC], F32, tag="d", name=f"d_ps{hl}")
                stile = {}

                def emit_scores(kb, hl):
                    h = 2 * p + hl
                    j = kb - sc * NSUB
                    off = j * P if j > 0 else 0
                    t_ = cyc.tile([P, SC], F32, tag="ps", name="score_ps")
                    nc.tensor.matmul(
                        t_[:, off:],
                        kT_sb[:, h, kb * P : (kb + 1) * P],
                        qt[hl][:, off:],
                        skip_group_check=True,
                    )
                    if j >= 0:
                        nc.vector.tensor_add(
                            t_[:, j * P : (j + 1) * P],
                            t_[:, j * P : (j + 1) * P],
                            tri_sb[:],
                        )
                    stile[(kb, hl)] = (t_, off)

                seq = [(kb, hl) for kb in range(nkb) for hl in range(2)]
                for s_ in seq[:LOOKAHEAD]:
                    emit_scores(*s_)
                for i, (kb, hl) in enumerate(seq):
                    h = 2 * p + hl
                    t_, off = stile.pop((kb, hl))
                    e = expp.tile([P, SC], BF16, tag="e", name="e")
                    nc.scalar.activation(
                        e[:, off:], t_[:, off:], AF.Exp, scale=inv_sqrt_hd
                    )
                    nc.tensor.matmul(
                        o_ps[hl][:, off:],
                        v_sb[:, kb, h * HD : (h + 1) * HD],
                        e[:, off:],
                        start=(kb == 0), stop=(kb == nkb - 1),
                        skip_group_check=True,
                    )
                    nc.tensor.matmul(
                        d_ps[hl][:, off:],
                        ones_sb[:],
                        e[:, off:],
                        start=(kb == 0), stop=(kb == nkb - 1),
                        skip_group_check=True,
                    )
                    if i + LOOKAHEAD < len(seq):
                        emit_scores(*seq[i + LOOKAHEAD])
                        if fillers and i % 2 == 1:
                            fillers.popleft()()
                    elif fillers:
                        fillers.popleft()()
                for hl in range(2):
                    h = 2 * p + hl
                    recip = scr.tile([P, SC], F32, tag="recip", name="recip")
                    nc.vector.reciprocal_approx_fast(recip[:], d_ps[hl][:])
                    nc.vector.tensor_mul(
                        outT[:, h, sc * SC : (sc + 1) * SC],
                        o_ps[hl][:], recip[:],
                    )

            def make_wo_blocks(sc, tail=False):
                """One closure per (s-tile, d-chunk) block of the fused wo
                projection for chunk sc: 4 accumulating matmuls over the
                head dim, PSUM->SBUF bf16 copy (alternating DVE/Pool), and
                the output DMA.  Tail blocks rotate through 3 PSUM pools."""
                work = []
                shared = {}
                for sti in range(NSUB):
                    st = sc * NSUB + sti
                    for dc in range(NDC):
                        dsl = slice(dc * SC, (dc + 1) * SC)
                        idx = len(work)

                        def blk(st=st, dc=dc, dsl=dsl, idx=idx):
                            if tail:
                                pool, tag = (
                                    (yps, "y"), (ops, "o"), (dps, "d")
                                )[idx % 3]
                            else:
                                pool, tag = yps, "y"
                            y_ps = pool.tile(
                                [P, SC], F32, tag=tag, name="wo_ps"
                            )
                            for h in range(H_LOC):
                                nc.tensor.matmul(
                                    y_ps[:],
                                    outT[:, h, st * P : (st + 1) * P],
                                    wo_sb[:, h, dsl],
                                    start=(h == 0), stop=(h == H_LOC - 1),
                                )
                            if dc == 0:
                                shared["ysb"] = yop.tile(
                                    [P, NDC * SC], BF16, tag="ysb", name="y_sb"
                                )
                            y_sb = shared["ysb"]
                            qtr = slice(dc * SC, (dc + 1) * SC)
                            nc.vector.tensor_copy(y_sb[:, qtr], y_ps[:])
                            if dc == NDC - 1:
                                nc.sync.dma_start(
                                    y[st * P : (st + 1) * P, :], y_sb[:]
                                )

                        work.append(blk)
                return work

            pending = deque()
            loaded = load_x(0, with_w=True)
            for sc in range(NQC):
                xt, cos_t, sin_t = loaded
                qT_t = qtp.tile([P, H_LOC, SC], BF16, tag="qT", name="qT_t")
                project_pair(0, sc, xt, cos_t, sin_t, qT_t, komajor=(sc == 0))
                if sc > 0 and sc + 1 < NQC:
                    loaded = load_x(sc + 1, with_w=False)
                attend(sc, 0, qT_t, pending)
                if sc == 0:
                    # wo is first needed by chunk-1 fillers; stream it on
                    # the (now idle) gpsimd ring behind the v weights, out
                    # of the warmup window.
                    nc.gpsimd.dma_start(wo_sb[:], woG[:])
                project_pair(1, sc, xt, cos_t, sin_t, qT_t)
                if sc == 0:
                    loaded = load_x(1, with_w=False)
                attend(sc, 1, qT_t, pending)
                if sc < NQC - 1:
                    pending.extend(make_wo_blocks(sc))
            while pending:
                pending.popleft()()
            for blk in make_wo_blocks(NQC - 1, tail=True):
                blk()

    nc.compile()
    return nc


def _build_core_kernel_legacy(causal: bool):
    """Baseline f32r kernel, kept as the non-causal fallback."""
    KO = D // P            # 16 contraction subtiles for projections
    NQC = S // SC          # 4 q-chunks
    NSUB = SC // P         # 4 128-blocks per chunk
    NST = S // P           # 16 s-tiles
    NHB = H_LOC // 2       # head pairs
    inv_sqrt_hd = 1.0 / float(np.sqrt(HD))

    nc = bacc.Bacc(None, target_bir_lowering=False)

    xT = nc.dram_tensor("xT", [D, S], F32R, kind="ExternalInput")
    wqkvT = nc.dram_tensor(
        "wqkvT", [H_LOC // 2, D, 6 * HD], F32R, kind="ExternalInput"
    )
    woT = nc.dram_tensor("woT", [HW, D], F32R, kind="ExternalInput")
    cosT = nc.dram_tensor("cosT", [HD, S], F32, kind="ExternalInput")
    sinT = nc.dram_tensor("sinT", [HD, S], F32, kind="ExternalInput")
    PT = nc.dram_tensor("PT", [HD, HD], F32R, kind="ExternalInput")
    ones = nc.dram_tensor("ones", [P, P], F32R, kind="ExternalInput")
    if causal:
        maskT = nc.dram_tensor("maskT", [SC, SC], F32, kind="ExternalInput")
    else:
        maskT = nc.dram_tensor("maskT", [S, S], F32, kind="ExternalInput")
    y = nc.dram_tensor("y", [S, D], F32, kind="ExternalOutput")

    xT_r = xT.rearrange("(ko ki) s -> ki ko s", ki=P)
    wqkvT_r = wqkvT.rearrange("hb (ko ki) c -> hb ki ko c", ki=P)
    woT_r = woT.rearrange("(h ki) d -> ki h d", ki=P)

    with tile.TileContext(nc) as tc:
        with (
            tc.tile_pool(name="persist", bufs=1) as persist,
            tc.tile_pool(name="wpool", bufs=1) as wpool,
            tc.tile_pool(name="kvq", bufs=1) as kvq,
            tc.tile_pool(name="xa", bufs=1) as xa,
            tc.tile_pool(name="cs", bufs=2) as cspool,
            tc.tile_pool(name="scr", bufs=2) as scr,
            tc.tile_pool(name="exps", bufs=4) as expp,
            tc.tile_pool(name="outq", bufs=2) as outqp,
            tc.tile_pool(name="yo", bufs=4) as yop,
            tc.tile_pool(name="gm", bufs=3) as gmp,
            tc.tile_pool(name="ps", bufs=3, space="PSUM") as cyc,
            tc.tile_pool(name="ops", bufs=2, space="PSUM") as ops,
            tc.tile_pool(name="dps", bufs=2, space="PSUM") as dps,
            tc.tile_pool(name="yps", bufs=1, space="PSUM") as yps,
        ):
            pt_sb = persist.tile([P, HD], F32R)
            nc.sync.dma_start(pt_sb[:], PT[:])
            ones_sb = persist.tile([P, P], F32R)
            nc.sync.dma_start(ones_sb[:], ones[:])
            if causal:
                mask_sb = persist.tile([P, NSUB, SC], F32)
                nc.sync.dma_start(
                    mask_sb[:], maskT.rearrange("(j ki) q -> ki j q", ki=P)
                )

            def load_chunk(sc):
                ssl = slice(sc * SC, (sc + 1) * SC)
                xt = xa.tile([P, KO, SC], F32R, tag="xt")
                for ko in range(KO):
                    nc.sync.dma_start(xt[:, ko], xT_r[:, ko, ssl])
                cos_t = cspool.tile([P, SC], F32, tag="cos")
                sin_t = cspool.tile([P, SC], F32, tag="sin")
                nc.sync.dma_start(cos_t[:], cosT[:, ssl])
                nc.sync.dma_start(sin_t[:], sinT[:, ssl])
                return xt, cos_t, sin_t

            preloaded = None
            for hb in range(NHB):
                if hb == 0:
                    preloaded = load_chunk(0)
                w_sb = wpool.tile([P, KO, 6 * HD], F32R, tag="w")
                for ko in range(KO):
                    nc.sync.dma_start(
                        w_sb[:, ko, :], wqkvT_r[hb, :, ko, :]
                    )
                wo_sb = wpool.tile([P, 2, D], F32R, tag="wo")
                for hl in range(2):
                    nc.sync.dma_start(
                        wo_sb[:, hl], woT_r[:, hb * 2 + hl, :]
                    )

                kT_sb = kvq.tile([P, 2, S], F32R, tag="kT")
                v_sb = kvq.tile([P, NST, 2 * HD], F32R, tag="v")
                deferred = not causal
                qT_full = (
                    kvq.tile([P, 2, S], F32R, tag="qT", name="qT_full") if deferred else None
                )

                def project_chunk(sc, loaded):
                    if deferred:
                        qT_dst = qT_full
                    else:
                        qT_dst = outqp.tile([P, 2, SC], F32R, tag="qTc")
                    ssl = slice(sc * SC, (sc + 1) * SC)
                    xt, cos_t, sin_t = loaded

                    for hl in range(2):
                        for t in range(2):  # 0=q, 1=k
                            wcols = slice(
                                (2 * hl + t) * HD, (2 * hl + t + 1) * HD
                            )
                            ps = cyc.tile([P, SC], F32, tag="ps")
                            for ko in range(KO):
                                nc.tensor.matmul(
                                    ps[:],
                                    w_sb[:, ko, wcols],
                                    xt[:, ko],
                                    start=(ko == 0),
                                    stop=(ko == KO - 1),
                                )
                            plain = scr.tile([P, SC], F32R, tag="plain")
                            nc.scalar.copy(plain[:], ps[:])
                            rot = cyc.tile([P, SC], F32, tag="ps")
                            nc.tensor.matmul(rot[:], pt_sb[:], plain[:])
                            if t == 0:
                                dst = (
                                    qT_dst[:, hl, ssl]
                                    if deferred
                                    else qT_dst[:, hl, :]
                                )
                            else:
                                dst = kT_sb[:, hl, ssl]
                            pc = scr.tile([P, SC], F32, tag="pc")
                            nc.gpsimd.tensor_mul(pc[:], plain[:], cos_t[:])
                            tmp2 = scr.tile([P, SC], F32, tag="tmp2")
                            nc.vector.tensor_mul(tmp2[:], rot[:], sin_t[:])
                            nc.vector.tensor_add(dst, pc[:], tmp2[:])

                    for sti in range(NSUB):
                        st = sc * NSUB + sti
                        lsl = slice(sti * P, (sti + 1) * P)
                        psv = cyc.tile([P, 2 * HD], F32, tag="ps")
                        for ko in range(KO):
                            nc.tensor.matmul(
                                psv[:],
                                xt[:, ko, lsl],
                                w_sb[:, ko, 4 * HD : 6 * HD],
                                start=(ko == 0),
                                stop=(ko == KO - 1),
                            )
                        vdst = v_sb[:, st, :]
                        nc.scalar.copy(vdst, psv[:])
                    return qT_dst

                def attend_chunk(qc, qT_cur, outT_qc, fillers):
                    nkb = (qc + 1) * NSUB if causal else NST
                    qt = {}
                    o_ps = {}
                    d_ps = {}
                    for hl in range(2):
                        qt[hl] = (
                            qT_cur[:, hl, qc * SC : (qc + 1) * SC]
                            if deferred
                            else qT_cur[:, hl, :]
                        )
                        o_ps[hl] = ops.tile([P, SC], F32, tag="o", name=f"o_ps{hl}")
                        d_ps[hl] = dps.tile([P, SC], F32, tag="d", name=f"d_ps{hl}")
                    stile = {}

                    def emit_scores(kb, hl):
                        t_ = cyc.tile([P, SC], F32, tag="ps")
                        nc.tensor.matmul(
                            t_[:],
                            kT_sb[:, hl, kb * P : (kb + 1) * P],
                            qt[hl],
                            skip_group_check=True,
                        )
                        if causal:
                            j = kb - qc * NSUB
                            if j >= 0:
                                w_ = P * (j + 1)
                                nc.vector.tensor_add(
                                    t_[:, :w_], t_[:, :w_],
                                    mask_sb[:, j, :w_],
                                )
                        else:
                            if hl == 0:
                                mt = gmp.tile([P, SC], F32, tag="mt")
                                nc.sync.dma_start(
                                    mt[:],
                                    maskT[
                                        kb * P : (kb + 1) * P,
                                        qc * SC : (qc + 1) * SC,
                                    ],
                                )
                                stile[("m", kb)] = mt
                            nc.vector.tensor_add(
                                t_[:], t_[:], stile[("m", kb)][:]
                            )
                        stile[(kb, hl)] = t_

                    seq = [(kb, hl) for kb in range(nkb) for hl in range(2)]
                    for kb, hl in seq[:LOOKAHEAD]:
                        emit_scores(kb, hl)
                    for i, (kb, hl) in enumerate(seq):
                        e = expp.tile([P, SC], F32R, tag="e")
                        nc.scalar.activation(
                            e[:], stile.pop((kb, hl))[:], AF.Exp,
                            scale=inv_sqrt_hd,
                        )
                        nc.tensor.matmul(
                            o_ps[hl][:],
                            v_sb[:, kb, hl * HD : (hl + 1) * HD],
                            e[:],
                            start=(kb == 0),
                            stop=(kb == nkb - 1),
                            skip_group_check=True,
                        )
                        nc.tensor.matmul(
                            d_ps[hl][:],
                            ones_sb[:],
                            e[:],
                            start=(kb == 0),
                            stop=(kb == nkb - 1),
                            skip_group_check=True,
                        )
                        if i + LOOKAHEAD < len(seq):
                            emit_scores(*seq[i + LOOKAHEAD])
                            if fillers and i % 2 == 1:
                                fillers.popleft()()
                        elif fillers:
                            fillers.popleft()()
                    for hl in range(2):
                        recip = scr.tile([P, SC], F32, tag="recip")
                        nc.vector.reciprocal_approx_fast(
                            recip[:], d_ps[hl][:]
                        )
                        nc.vector.tensor_mul(
                            outT_qc[:, hl, :], o_ps[hl][:], recip[:]
                        )

                def make_out_fillers(hb, qc, outT_qc):
                    work = []
                    for sti in range(NSUB):
                        st = qc * NSUB + sti
                        stsl = slice(sti * P, (sti + 1) * P)
                        for dc in range(D // SC):
                            dsl = slice(dc * SC, (dc + 1) * SC)

                            def blk(st=st, stsl=stsl, dsl=dsl):
                                y_ps = yps.tile([P, SC], F32, tag="y")
                                for hl in range(2):
                                    nc.tensor.matmul(
                                        y_ps[:],
                                        outT_qc[:, hl, stsl],
                                        wo_sb[:, hl, dsl],
                                        start=(hl == 0),
                                        stop=(hl == 1),
                                    )
                                y_sb = yop.tile([P, SC], F32, tag="ysb")
                                nc.vector.tensor_copy(y_sb[:], y_ps[:])
                                ydst = y[st * P : (st + 1) * P, dsl]
                                if hb == 0:
                                    nc.sync.dma_start(ydst, y_sb[:])
                                else:
                                    nc.gpsimd.dma_start(
                                        ydst, y_sb[:],
                                        accum_op=mybir.AluOpType.add,
                                    )

                            work.append(blk)
                    return work

                pending = deque()
                if causal:
                    for sc in range(NQC):
                        loaded = preloaded if sc == 0 and preloaded else load_chunk(sc)
                        preloaded = None
                        qT_cur = project_chunk(sc, loaded)
                        outT_qc = outqp.tile([P, 2, SC], F32R, tag="outq")
                        attend_chunk(sc, qT_cur, outT_qc, pending)
                        pending.extend(make_out_fillers(hb, sc, outT_qc))
                else:
                    for sc in range(NQC):
                        loaded = preloaded if sc == 0 and preloaded else load_chunk(sc)
                        preloaded = None
                        project_chunk(sc, loaded)
                    for qc in range(NQC):
                        outT_qc = outqp.tile([P, 2, SC], F32R, tag="outq")
                        attend_chunk(qc, qT_full, outT_qc, pending)
                        pending.extend(make_out_fillers(hb, qc, outT_qc))
                while pending:
                    pending.popleft()()

    nc.compile()
    return nc


_NC_CACHE = {}


def _get_nc(key):
    if key not in _NC_CACHE:
        if key == "v2":
            _NC_CACHE[key] = _build_core_kernel_v2()
        else:
            _NC_CACHE[key] = _build_core_kernel_legacy(causal=False)
    return _NC_CACHE[key]


def _rope_perm_T() -> np.ndarray:
    # rotate_half as a matrix: (P_rh @ q)[d] = -q[d+HD/2] for d < HD/2,
    # q[d-HD/2] otherwise.  Returns P_rh.T for use as matmul lhsT.
    P_rh = np.zeros((HD, HD), dtype=np.float32)
    half = HD // 2
    for i in range(half):
        P_rh[i, half + i] = -1.0
        P_rh[half + i, i] = 1.0
    return np.ascontiguousarray(P_rh.T)


def _is_causal(m: np.ndarray) -> bool:
    tril = np.tril(np.ones((S, S), dtype=bool))
    if not np.all(m[tril] == 0.0):
        return False
    upper = m[~tril]
    return bool(upper.size == 0 or np.all(upper <= -1.0e8))


# module-level: results of the last traced run (for test harnesses)
last_exec_time_ns = None
last_profile_json = None


def _run(nc, in_maps, _trace):
    kw = dict(trace=True) if _trace else {}
    res = run_bass_kernel_spmd(
        nc, in_maps, core_ids=list(range(N_CORES)), **kw
    )
    global last_exec_time_ns, last_profile_json
    last_exec_time_ns = res.exec_time_ns
    last_profile_json = res.profile_json
    return res


def _kernel_v2(x, cos, sin, m2d, wq, wk, wv, wo, _trace):
    scale = np.float32(np.sqrt(HD))
    triT = np.ascontiguousarray((m2d[:P, :P] * scale).T).astype(np.float32)
    cosT = np.ascontiguousarray(cos.T, dtype=np.float32)
    sinT = np.ascontiguousarray(sin.T, dtype=np.float32)
    ptT = _rope_perm_T().astype(NPBF)
    ones = np.ones((P, P), dtype=NPBF)
    # chunk-major x: [ki, chunk, ko, s] = xT[ko*128+ki, chunk*512+s]
    xTc = [
        np.ascontiguousarray(
            x[b].T.reshape(D // P, P, S // SC, SC).transpose(1, 2, 0, 3)
        ).astype(NPBF)
        for b in range(B)
    ]

    in_maps = []
    for c in range(N_CORES):
        b = c // (N_CORES // B)
        hg = c % (N_CORES // B)
        heads = [hg * H_LOC + i for i in range(H_LOC)]
        cols = []
        for w_ in (wq, wk):
            for h in heads:
                cols.append(w_[h * HD : (h + 1) * HD].T)
        wqkT = np.concatenate(cols, axis=1)          # [D, 1024]
        wvT = np.concatenate(
            [wv[h * HD : (h + 1) * HD].T for h in heads], axis=1
        )                                            # [D, 512]
        rows = slice(hg * HW, (hg + 1) * HW)
        woT = np.ascontiguousarray(wo[:, rows].T)    # [512, D]
        # partition-major: [ki, ko/h, cols] with contiguous per-ki runs
        wqkG = np.ascontiguousarray(
            wqkT.reshape(D // P, P, 8 * HD).transpose(1, 0, 2)
        ).astype(NPBF)
        wvG = np.ascontiguousarray(
            wvT.reshape(D // P, P, 4 * HD).transpose(1, 0, 2)
        ).astype(NPBF)
        woG = np.ascontiguousarray(
            woT.reshape(H_LOC, P, D).transpose(1, 0, 2)
        ).astype(NPBF)
        in_maps.append(
            {
                "xTc": xTc[b],
                "wqkG": wqkG,
                "wvG": wvG,
                "woG": woG,
                "cosT": cosT,
                "sinT": sinT,
                "PT": ptT,
                "ones": ones,
                "tri": triT,
            }
        )

    res = _run(_get_nc("v2"), in_maps, _trace)

    out = np.empty((B, S, D), dtype=np.float32)
    gs = N_CORES // B
    for b in range(B):
        acc = res.results[b * gs]["y"].astype(np.float32)
        for g in range(1, gs):
            acc = acc + res.results[b * gs + g]["y"].astype(np.float32)
        out[b] = acc
    return out


def _kernel_legacy(x, cos, sin, m2d, wq, wk, wv, wo, _trace):
    scale = np.float32(np.sqrt(HD))
    maskT = np.ascontiguousarray((m2d * scale).T)
    cosT = np.ascontiguousarray(cos.T, dtype=np.float32)
    sinT = np.ascontiguousarray(sin.T, dtype=np.float32)
    ptT = _round_f32r(_rope_perm_T())
    ones = np.ones((P, P), dtype=np.float32)

    xT = [_round_f32r(x[b].T) for b in range(B)]

    in_maps = []
    for c in range(N_CORES):
        b = c // (N_CORES // B)
        hg = c % (N_CORES // B)
        rows = slice(hg * HW, (hg + 1) * HW)
        packs = []
        for hbp in range(H_LOC // 2):
            cols = []
            for hl in range(2):
                h = hg * H_LOC + hbp * 2 + hl
                cols.append(wq[h * HD : (h + 1) * HD].T)
                cols.append(wk[h * HD : (h + 1) * HD].T)
            for hl in range(2):
                h = hg * H_LOC + hbp * 2 + hl
                cols.append(wv[h * HD : (h + 1) * HD].T)
            packs.append(np.concatenate(cols, axis=1))
        wqkvT = np.stack(packs)
        in_maps.append(
            {
                "xT": xT[b],
                "wqkvT": _round_f32r(wqkvT),
                "woT": _round_f32r(np.ascontiguousarray(wo[:, rows].T)),
                "cosT": cosT,
                "sinT": sinT,
                "PT": ptT,
                "ones": ones,
                "maskT": maskT.astype(np.float32),
            }
        )

    res = _run(_get_nc("legacy"), in_maps, _trace)

    out = np.empty((B, S, D), dtype=np.float32)
    gs = N_CORES // B
    for b in range(B):
        acc = res.results[b * gs]["y"].astype(np.float32).copy()
        for g in range(1, gs):
            acc += res.results[b * gs + g]["y"]
        out[b] = acc
    return out


def kernel(x, cos, sin, mask, wq, wk, wv, wo, _trace=False):
    x = np.asarray(x, dtype=np.float32)
    cos = np.asarray(cos, dtype=np.float32)
    sin = np.asarray(sin, dtype=np.float32)
    mask = np.asarray(mask, dtype=np.float32)
    wq = np.asarray(wq, dtype=np.float32)
    wk = np.asarray(wk, dtype=np.float32)
    wv = np.asarray(wv, dtype=np.float32)
    wo = np.asarray(wo, dtype=np.float32)

    m2d = mask.reshape(S, S)
    if _is_causal(m2d):
        return _kernel_v2(x, cos, sin, m2d, wq, wk, wv, wo, _trace)
    return _kernel_legacy(x, cos, sin, m2d, wq, wk, wv, wo, _trace)
